# revision 105
# baseline (speedup 1.0000x reference)
"""Trainium2 Bass kernel for nn_AttentionLayer (B=4, T=2048, C=1024, H=16, D=64).

Sharding: 8 cores = 4 batches x 2 head-groups (8 heads each). Each core
computes a partial y[b] = out_g @ Wo_g^T; host sums the two group partials
per batch and transposes back.

Single fused pipeline engineered around the per-engine floors: ScalarE exp
(256 x ~1.04us), PE matmul out-element cost (scores+PV+projections ~278us),
and DVE (mask multiplies + rotary + copies ~263us). Design:
  - bf16 throughout; k/q projected per 512-column chunk with xpos rotary
    fused on DVE (+GPSIMD for one multiply), stored as bf16 kf (full T) and
    per-quarter double-buffered qf.
  - scoresT[tk,tq] per head-pair in a double-buffered PSUM pair; exp on
    ScalarE (constant -40 bias, 1/32 scale - no row-max pass needed).
  - mask applied POST-exp as a bf16 {0,1} multiply on DVE (2x mode, mask
    broadcast over the head pair). Its emission is deferred until the pv
    pop SKEW slots later, so a TT stalled on a mask DMA never wedges the
    in-order DVE queue ahead of rotary work, and mask loads get ~20us of
    deadline slack. (GPSIMD cannot touch PSUM on real HW, so all PSUM
    reads - copies, normalize - stay on DVE/ACT.)
  - transposed PV: out[q,0:66] += pr[:,e,qtile].T @ vaug[tk,h,0:66] (cols
    64:66 ones = softmax denominator); 16 accumulators (qt-major: s=qt*2+e)
    packed in 3 PSUM banks with per-bank start/stop flags. Epilogue: one
    reciprocal + ONE broadcast tensor_tensor normalizes 7 accumulators at
    a stroke; PE transposes read adjacent (e0,e1) pairs; Wo per 128-row
    output block, output staged through a 4-deep ysb ring.
  - DMA discipline: every dma_start pays ~625ns on the single HWDGE
    descriptor-gen device and transfers drain one DMA_ENGINES queue in
    issue order, so the prologue issues exactly in dependency order
    (x0/wk interleaved 2-chunk pieces, ktabs0, wq, qtabs0, x1, ktabs1,
    x2, ktabs2, wv, x3, ktabs3, masks, ident, wo) and bulk loads never
    preempt the first-scores path.
  - quarter-0 carries all k/v projections (every later quarter re-sweeps
    all tk): injected into attention slots sequenced by DMA arrival, with
    k-chunk generators force-drained 3 slots before first use to hide the
    8-MM chain + rotary latency. The pv backlog tapers near quarter ends
    so deferred work doesn't dump into the next quarter's first slots.
  - warm-up pacer matmuls ride out the PE p-state ramp (post-idle matmuls
    run at half clock for ~3us) during the initial weight DMA.
"""

import numpy as np
import ml_dtypes

B, T, C, H, D = 4, 2048, 1024, 16, 64
G = 2                 # head groups (tensor parallel)
NCORES = B * G
CG = C // G           # 512 channels per group
JT = CG // 128        # 4 head-pairs per core
CCH = C // 128        # 8 contraction chunks
TT = T // 128         # 16 tk-tiles
NQ = 4                # tq quarters of 512
THETA = 10000.0
SCALE_BASE = 512.0

# scheduling knobs
SKEW = 19             # pv matmuls trail the scores/exp/mask stream (slots)
PR_BUFS = 23
WARMUP_PACERS = 26
PACER_EVERY = 0       # if >0: one pacer MM every N slots in quarters 1-3

_CACHE = {}


def _rot_tables_np():
    inv_freq = 1.0 / (THETA ** (np.arange(0, D, 2, dtype=np.float32) / D))
    seq = np.arange(T, dtype=np.float32)
    freqs = seq[:, None] * inv_freq[None, :]
    freqs = np.repeat(freqs, 2, axis=-1)                    # [T, D]
    base = (np.arange(0, D, 2, dtype=np.float32) + 0.4 * D) / (1.4 * D)
    power = (seq - T // 2) / SCALE_BASE
    scale = base[None, :] ** power[:, None]
    scale = np.repeat(scale, 2, axis=-1)                    # [T, D]
    return np.cos(freqs), np.sin(freqs), scale.astype(np.float32)


def _build_bass():
    import concourse.bass as bass
    import concourse.bacc as bacc
    import concourse.mybir as mybir
    import concourse.tile as tile
    from concourse.bass import ts, ds

    fp32 = mybir.dt.float32
    bf16 = mybir.dt.bfloat16
    MUL = mybir.AluOpType.mult
    ADD = mybir.AluOpType.add
    EXP = mybir.ActivationFunctionType.Exp

    nc = bacc.Bacc(None)

    xT = nc.dram_tensor("xT", [C, T], bf16, kind="ExternalInput")
    wq = nc.dram_tensor("wq", [C, CG], bf16, kind="ExternalInput")
    wk = nc.dram_tensor("wk", [C, CG], bf16, kind="ExternalInput")
    wv = nc.dram_tensor("wv", [C, CG], bf16, kind="ExternalInput")
    wo = nc.dram_tensor("wo", [CG, C], bf16, kind="ExternalInput")
    qcos = nc.dram_tensor("qcos", [128, T], fp32, kind="ExternalInput")
    qsin = nc.dram_tensor("qsin", [128, T], fp32, kind="ExternalInput")
    kcos = nc.dram_tensor("kcos", [128, T], fp32, kind="ExternalInput")
    ksin = nc.dram_tensor("ksin", [128, T], fp32, kind="ExternalInput")
    maskB = nc.dram_tensor("maskB", [T, T], bf16, kind="ExternalInput")
    ident_d = nc.dram_tensor("ident", [128, 128], bf16, kind="ExternalInput")
    yT = nc.dram_tensor("yT", [C, T], fp32, kind="ExternalOutput")

    xT_r = xT.rearrange("(cc p) t -> p cc t", p=128)        # [128, 8, T]
    maskB_r = maskB.rearrange("(tk p) q -> p tk q", p=128)  # [128, 16, T]
    SWAPM = [i + 1 - 2 * (i % 2) for i in range(32)]

    # pv accumulator group = one head-pair (jt): 8 accumulators [128, 66]
    # in 2 PSUM banks (pvA rows 0..6, pvB row 0 = s7). The accumulate
    # zero-region is a whole bank, so only the chronologically-first MM per
    # bank starts the group and the last stops it.
    PV_START = {0, 7}
    PV_STOP = {5, 7}
    MASK_ON_GPSIMD = set()  # (quarter, slot) pairs: mask-mult on Pool

    with tile.TileContext(nc) as tc:
        with (
            tc.tile_pool(name="persist", bufs=1) as persist,
            tc.tile_pool(name="xp", bufs=4) as xp,
            tc.tile_pool(name="ktabp", bufs=4) as ktabp,
            tc.tile_pool(name="qtabp", bufs=1) as qtabp,
            tc.tile_pool(name="maskp", bufs=3) as maskp,
            tc.tile_pool(name="prp", bufs=PR_BUFS) as prp,
            tc.tile_pool(name="rotp", bufs=3) as rotp,
            tc.tile_pool(name="stgp", bufs=2) as stgp,
            tc.tile_pool(name="recp", bufs=6) as recp,
            tc.tile_pool(name="ysbp", bufs=4) as ysbp,
            tc.tile_pool(name="scp", bufs=2, space="PSUM") as scp,
            tc.tile_pool(name="pvp", bufs=2, space="PSUM") as pvp,
            tc.tile_pool(name="tailp", bufs=2, space="PSUM") as tailp,
        ):
            kf = persist.tile([128, JT, T], bf16, tag="kf")
            qfq = persist.tile([128, 2, JT, 512], bf16, tag="qfq")
            vaug = persist.tile([128, TT, 8, 66], bf16, tag="vaug")
            wq_sb = persist.tile([128, CCH, CG], bf16, tag="wq")
            # wk shares the mask ring: k projections finish in quarter 0,
            # after which its 8KB slot recycles into mask buffers.
            wk_sb = maskp.tile([128, CCH, CG], bf16, tag="mask", name="wk_sb")
            wv_sb = persist.tile([128, CCH, CG], bf16, tag="wv")
            wo_sb = persist.tile([128, JT, C], bf16, tag="wo")
            oth = persist.tile([128, JT, 512], bf16, tag="oth")
            ident = persist.tile([128, 128], bf16, tag="ident")
            bias_m40 = persist.tile([128, 1], fp32, tag="biasm40")
            dmy_a = persist.tile([128, 128], bf16, tag="dmya")
            dmy_b = persist.tile([128, 256], bf16, tag="dmyb")
            dmy_s = persist.tile([128, 1], fp32, tag="dmys")

            # ---- t=0 warm-up: pacers ride out the p-state ramp + DMA head;
            # a dummy activation pulls the exp table load forward.
            nc.vector.memset(dmy_a[:], 0.0)
            nc.vector.memset(dmy_b[:], 0.0)
            nc.vector.memset(bias_m40[:], -40.0)
            nc.vector.memset(vaug[:, :, :, 64:66], 1.0)
            nc.scalar.activation(dmy_s[:], bias_m40[:], EXP,
                                 bias=bias_m40[:, :], scale=0.0)
            pacer_ps = tailp.tile([128, 512], fp32, tag="tail")
            for _ in range(WARMUP_PACERS):
                nc.tensor.matmul(pacer_ps[:, 0:256], dmy_a[:], dmy_b[:],
                                 start=True, stop=True)

            def pacer(n):
                pps = tailp.tile([128, 512], fp32, tag="tail")
                for _ in range(n):
                    nc.tensor.matmul(pps[:, 0:256], dmy_a[:], dmy_b[:],
                                     start=True, stop=True)

            # ---- weight DMA (k path first - it gates the first scores) ---
            # Two half-granularity DMAs per weight: each dma_start pays
            # ~625ns on the single HWDGE descriptor-gen device, so fewer,
            # bigger transfers win; halves keep the prelude MM chain
            # streaming. Bulk loads go through gpsimd (SWDGE) which has its
            # own desc-gen path and leaves HWDGE free for the urgent loads.
            def wdma(w_sb, w_dr, n=2):
                r = w_dr.rearrange("(cc p) j -> p cc j", p=128)
                step = CCH // n
                for i in range(0, CCH, step):
                    nc.sync.dma_start(out=w_sb[:, i:i + step, :],
                                      in_=r[:, i:i + step, :])

            def load_x(tcx):
                tsl = ds(tcx * 512, 512)
                xtc = xp.tile([128, CCH, 512], bf16, tag="x")
                nc.sync.dma_start(out=xtc[:], in_=xT_r[:, :, tsl])
                return xtc

            def load_tabs(tcx, which, eng=None):
                tsl = ds(tcx * 512, 512)
                srcs = {"q": (("tqc", qcos), ("tqs", qsin)),
                        "k": (("tkc", kcos), ("tks", ksin))}[which]
                pool = qtabp if which == "q" else ktabp
                eng = eng or nc.sync
                tabs = []
                for nm, dr in srcs:
                    t = pool.tile([128, 512], fp32, tag=nm)
                    eng.dma_start(out=t[:], in_=dr[:, tsl])
                    tabs.append(t)
                return tabs

            def rotary(ps, tabs, dst, eng2=None):
                # dst = ps*cos + pairswap(ps)*sin   (sign folded into sin)
                swp = rotp.tile([128, 512], fp32, tag="rt")
                nc.vector.stream_shuffle(swp[:], ps[:], SWAPM)
                t1 = rotp.tile([128, 512], fp32, tag="rt")
                nc.vector.tensor_tensor(t1[:], ps[:], tabs[0][:], MUL)
                t2 = rotp.tile([128, 512], fp32, tag="rt")
                (eng2 or nc.gpsimd).tensor_tensor(t2[:], swp[:], tabs[1][:], MUL)
                nc.vector.tensor_tensor(dst, t1[:], t2[:], ADD)

            # ---- prelude: x resident (all 4 chunks), k0/q0 for jt0 only;
            # everything else is injected into attention slots.
            held = {}

            def load_mask(q, half, eng=None):
                m = maskp.tile([128, 8, 512], bf16, tag="mask")
                (eng or nc.sync).dma_start(
                    out=m[:], in_=maskB_r[:, half * 8:half * 8 + 8,
                                          ds(q * 512, 512)])
                held[("mk", q, half)] = m

            # All prologue DMAs ride the sync HWDGE queue; transfers drain
            # the single DMA_ENGINES device in exactly this order. Sequence
            # by downstream need: first-scores path (x0,wk,ktabs0,qtabs0),
            # then mask half 0 (unblocks the slot-0 mask multiply on DVE),
            # x1 + wq, wv (v-tile pieces get pumped from slot ~1), k-tabs
            # and x chunks interleaved by deadline, mask half 1 before jt0's
            # tkt-8 multiply, wo/ident last.
            # interleave x0/wk piece DMAs so the k0 chain streams cc-by-cc
            xtc0 = xp.tile([128, CCH, 512], bf16, tag="x", name="xtc0")
            wk_r = wk.rearrange("(cc p) j -> p cc j", p=128)
            for i in range(0, CCH, 2):
                nc.sync.dma_start(out=xtc0[:, i:i + 2, :],
                                  in_=xT_r[:, i:i + 2, ds(0, 512)])
                nc.sync.dma_start(out=wk_sb[:, i:i + 2, :],
                                  in_=wk_r[:, i:i + 2, :])
            xall = [xtc0]
            ktabs = {0: load_tabs(0, "k")}
            wdma(wq_sb, wq, n=4)
            qtabs0 = load_tabs(0, "q")
            xall += [load_x(1)]
            ktabs[1] = load_tabs(1, "k")
            xall += [load_x(2)]
            ktabs[2] = load_tabs(2, "k")
            nc.sync.dma_start(
                out=wv_sb[:], in_=wv.rearrange("(cc p) j -> p cc j", p=128))
            xall += [load_x(3)]
            ktabs[3] = load_tabs(3, "k")
            load_mask(0, 0)
            load_mask(0, 1)
            nc.sync.dma_start(out=ident[:], in_=ident_d[:])
            nc.sync.dma_start(
                out=wo_sb[:], in_=wo.rearrange("(cc p) j -> p cc j", p=128))

            def proj_gen(w_sb, tcx, jt, fini):
                # generator piece: ~2 matmuls per step so injected work
                # never blocks the in-order PE stream for long
                ps = tailp.tile([128, 512], fp32, tag="tail")
                for h in range(4):
                    for cc in (2 * h, 2 * h + 1):
                        nc.tensor.matmul(ps[:], w_sb[:, cc, ts(jt, 128)],
                                         xall[tcx][:, cc, :],
                                         start=(cc == 0),
                                         stop=(cc == CCH - 1))
                    yield
                fini(ps)

            def kproj_gen(tcx, jt, eng2=None):
                return proj_gen(
                    wk_sb, tcx, jt,
                    lambda ps: rotary(ps[:], ktabs[tcx],
                                      kf[:, jt, ds(tcx * 512, 512)], eng2))

            def qproj_gen(tcx, tabs, jt, eng2=None):
                def fini(ps):
                    t = tabs["qt"] if isinstance(tabs, dict) else tabs
                    rotary(ps[:], t, qfq[:, tcx % 2, jt, :], eng2)
                return proj_gen(wq_sb, tcx, jt, fini)

            def kproj(tcx, jt, eng2=None):
                for _ in kproj_gen(tcx, jt, eng2):
                    pass

            def qproj(tcx, tabs, jt, eng2=None):
                for _ in qproj_gen(tcx, tabs, jt, eng2):
                    pass

            # jt0's chunk-0 projections in the prologue (k chain first: wk+x0
            # land before wq); jt1-3's ride early quarter-0 slots, which have
            # PE slack until PV kicks in at slot SKEW. Pacers bridge the PE
            # gap between the k chain and wq's arrival so the p-state ramp
            # never resets.
            kproj(0, 0, eng2=nc.vector)
            # the wq transfer is still in flight here; fill the window with
            # jt1-3's chunk-0 k chains instead of dummy pacers (their
            # rotaries also hide the first-scores wait on wq/qtabs0).
            kproj(0, 1, eng2=nc.vector)
            kproj(0, 2, eng2=nc.vector)
            kproj(0, 3, eng2=nc.vector)
            qproj(0, qtabs0, 0, eng2=nc.vector)

            # ---- injected side-work pieces -------------------------------
            def vtile_gen(tti_global):
                xt = xall[tti_global // 4]
                tsl = ts(tti_global % 4, 128)
                ps = tailp.tile([128, 8, 64], fp32, tag="tail")
                for h in range(4):
                    for cc in (2 * h, 2 * h + 1):
                        nc.tensor.matmul(ps[:, :, :], xt[:, cc, tsl],
                                         wv_sb[:, cc, :],
                                         start=(cc == 0),
                                         stop=(cc == CCH - 1))
                    yield
                nc.scalar.copy(vaug[:, tti_global, :, 0:64], ps[:, :, :])

            # ---- attention machinery -------------------------------------
            st = {"accs": None, "gkey": None, "y_written": 0}
            sideq = []                 # deferred small thunks (epilogue, wo)

            def acc_slice(accs, s):
                if s < 7:
                    return accs[0][:, s, :]
                return accs[1][:, s - 7, :]

            def epilogue(gkey, accs):
                tq4, jt = gkey
                rA = recp.tile([128, 8], fp32, tag="rec")
                nc.vector.reciprocal(rA[:, 0:7, None], accs[0][:, 0:7, 64:65])
                rB = recp.tile([128, 8], fp32, tag="rec")
                nc.vector.reciprocal(rB[:, 0:1, None], accs[1][:, 0:1, 64:65])

                # normalize all 7 pvA accumulators in ONE tensor_tensor with
                # the reciprocals broadcast along d (vs 7 serial TSPs); the
                # transposes then gather (s=qt, s=qt+4) as a strided slice.
                stg = stgp.tile([128, 8, 64], bf16, tag="stg")
                nc.vector.tensor_tensor(
                    stg[:, 0:7, :], accs[0][:, 0:7, 0:64],
                    rA[:, 0:7, None].broadcast_to((128, 7, 64)), MUL)
                nc.vector.tensor_scalar_mul(
                    stg[:, 7, :], accs[1][:, 0, 0:64], rB[:, 0:1])

                tb = tailp.tile([128, 4, 128], bf16, tag="tail")
                for qt in range(4):
                    nc.tensor.transpose(
                        tb[:, qt, :],
                        stg[:, 2 * qt:2 * qt + 2, :], ident[:])
                    nc.vector.tensor_copy(
                        oth[:, jt, ds(qt * 128, 128)], tb[:, qt, :])

            def wo_piece(tq4, jo):
                py = tailp.tile([128, 512], fp32, tag="tail")
                for cc in range(JT):
                    nc.tensor.matmul(py[:], wo_sb[:, cc, ts(jo, 128)],
                                     oth[:, cc, :],
                                     start=(cc == 0), stop=(cc == JT - 1))
                ysb = ysbp.tile([128, 512], fp32, tag="ysb")
                nc.vector.tensor_copy(ysb[:], py[:])
                nc.sync.dma_start(out=yT[ts(jo, 128), ds(tq4 * 512, 512)],
                                  in_=ysb[:])

            def ensure_group(gkey):
                if st["gkey"] == gkey:
                    return
                if st["gkey"] is not None:
                    epilogue(st["gkey"], st["accs"])
                    fin_tq4, fin_jt = st["gkey"]
                    if fin_jt == JT - 1:   # quarter's oth complete -> Wo
                        for jo in range(8):
                            sideq.append(
                                lambda tq4=fin_tq4, jo=jo: wo_piece(tq4, jo))
                pvA = pvp.tile([128, 7, 66], fp32, tag="pv")
                pvB = pvp.tile([128, 1, 66], fp32, tag="pv")
                st["accs"] = (pvA, pvB)
                st["gkey"] = gkey

            def emit_pv(pr, jt, tkt, gkey):
                ensure_group(gkey)
                for e in range(2):
                    h = jt * 2 + e
                    for qt in range(4):
                        s = qt * 2 + e
                        nc.tensor.matmul(
                            acc_slice(st["accs"], s),
                            pr[:, e, ts(qt, 128)],
                            vaug[:, tkt, h, 0:66],
                            start=(tkt == 0 and s in PV_START),
                            stop=(tkt == TT - 1 and s in PV_STOP),
                            skip_group_check=True)

            pvq = []                  # [(pr, jt, tkt, gkey, mb, r, slot), ...]
            workq = []                # [(key, generator)] fine-grained pieces

            def pop_pv():
                # DVE mask multiplies are emitted HERE, SKEW slots after the
                # exp: rotaries emitted near the exp slot never queue on DVE
                # behind a TT stalled on a mask DMA, and mask loads gain
                # ~20us of deadline slack. Pool-offloaded multiplies were
                # already emitted at exp time (their latency hides in the
                # skew and they never block DVE).
                pr, jt, tkt, gkey, mb, r, slot = pvq.pop(0)
                force(("v", tkt))           # vaug ready for pv
                if (gkey[0], slot) not in MASK_ON_GPSIMD:
                    nc.vector.tensor_tensor(
                        pr[:, :, :], pr[:, :, :],
                        mb[:, r, None, :].broadcast_to((128, 2, 512)), MUL)
                emit_pv(pr, jt, tkt, gkey)

            def pump(nsteps):
                while nsteps > 0 and workq:
                    key, g = workq[0]
                    try:
                        next(g)
                        nsteps -= 1
                    except StopIteration:
                        workq.pop(0)

            def force(key):
                # emission-order deadline: finish FIFO head pieces up to and
                # including `key` so dependent reads are emitted after writes
                while any(k == key for k, _ in workq):
                    k0, g = workq[0]
                    try:
                        next(g)
                    except StopIteration:
                        workq.pop(0)

            # ---- main loop ------------------------------------------------

            for tq4 in range(NQ):
                qsl = ds(tq4 * 512, 512)
                if tq4 > 0:
                    load_mask(tq4, 1)   # first half was prefetched

                inject = {}

                def add(slot, fn):
                    inject.setdefault(slot, []).append(fn)

                def addg(slot, key, mk):
                    # queue a generator piece at slot; tracked for deadlines
                    def starter():
                        workq.append((key, mk()))
                    add(slot, starter)

                def addk(slot, tcx, jt):
                    addg(slot, ("k", tcx, jt), lambda: kproj_gen(tcx, jt))

                def addq(slot, tcx, jt, tabs):
                    addg(slot, ("q", tcx, jt),
                         lambda: qproj_gen(tcx, tabs, jt))

                def addv(slot, i):
                    addg(slot, ("v", i), lambda: vtile_gen(i))

                if tq4 == 0:
                    # remaining k chunks per jt (deadline slot jt*16+4c),
                    # k0/q0 projections for jt 1-3 (deadline jt*16), all 16
                    # v tiles (deadline tt+SKEW), q quarter 1 late.
                    # workq is FIFO and pump pops the head only, so adds are
                    # sequenced by DMA arrival: x0/wq-based first, then the
                    # x1/x2/x3-gated chunks as close to their force slot as
                    # possible so a not-yet-landed DMA never wedges the head.
                    addk(0, 1, 0)                    # x1-based
                    addq(3, 0, 1, qtabs0)
                    addk(4, 2, 0)                    # x2-based
                    addq(7, 0, 2, qtabs0)
                    addv(7, 0)                       # wv-gated
                    addk(9, 3, 0)                    # x3-based
                    addv(9, 1)
                    addq(11, 0, 3, qtabs0)
                    addv(12, 2)
                    addv(13, 3)
                    addk(14, 1, 1)
                    addv(15, 4)
                    addk(16, 2, 1)
                    addv(17, 5)
                    addk(18, 3, 1)
                    addv(19, 6)
                    addv(20, 7)
                    addv(21, 8)                      # v8-11 (x2-based)
                    addv(22, 9)
                    addv(23, 10)
                    addv(24, 11)
                    addv(25, 12)                     # v12-15 (x3-based)
                    addv(26, 13)
                    addv(27, 14)
                    addv(28, 15)
                    addk(31, 1, 2)
                    addk(34, 2, 2)
                    addk(37, 3, 2)
                    addk(44, 1, 3)
                    addk(46, 2, 3)
                    addk(48, 3, 3)
                    add(50, lambda: held.__setitem__("qt", load_tabs(1, "q")))
                    for jt in range(JT):
                        addq(52 + 3 * jt, 1, jt, held)
                elif tq4 < NQ - 1:
                    add(24, lambda tq4=tq4: held.__setitem__(
                        "qt", load_tabs(tq4 + 1, "q")))
                    for jt in range(JT):
                        addq(28 + 3 * jt, tq4 + 1, jt, held)
                if tq4 < NQ - 1:
                    add(44, lambda tq4=tq4: load_mask(tq4 + 1, 0))

                for jt in range(JT):
                    for tkt in range(TT):
                        slot = jt * 16 + tkt
                        if tkt == 0:
                            force(("q", tq4, jt))   # qfq ready for scores
                        if tkt % 4 == 0:
                            force(("k", tkt // 4, jt))  # kf ready for scores
                        elif tkt % 4 == 1 and tkt < 13:
                            # drain the NEXT chunk 3 slots early so its
                            # 8-MM chain + rotary latency is hidden
                            force(("k", tkt // 4 + 1, jt))
                        ps = scp.tile([128, 2, 512], fp32, tag="sc")
                        mb = held[("mk", tq4, tkt // 8)]
                        r = tkt % 8
                        for e in range(2):
                            nc.tensor.matmul(
                                ps[:, e, :],
                                kf[ds(e * 64, 64), jt, ts(tkt, 128)],
                                qfq[ds(e * 64, 64), tq4 % 2, jt, :],
                                start=True, stop=True)
                        pump(2)
                        if tq4 == NQ - 1 and slot >= 36:
                            target = 1 if slot >= 52 else 4
                        elif slot >= 56:
                            target = 8   # taper into the quarter boundary
                        else:
                            target = SKEW
                        while len(pvq) >= target:
                            pop_pv()
                        pump(2)
                        pr = prp.tile([128, 2, 512], bf16, tag="pr")
                        nc.scalar.activation(pr[:, :, :], ps[:, :, :],
                                             EXP, bias=bias_m40[:, :],
                                             scale=0.03125)
                        if (tq4, slot) in MASK_ON_GPSIMD:
                            nc.gpsimd.tensor_tensor(
                                pr[:, :, :], pr[:, :, :],
                                mb[:, r, None, :].broadcast_to((128, 2, 512)),
                                MUL)
                        pvq.append((pr, jt, tkt, (tq4, jt), mb, r, slot))
                        for _ in range(2 if tq4 == NQ - 1 else 1):
                            if sideq:
                                sideq.pop(0)()
                        for fn in inject.get(slot, ()):
                            fn()
                        pump(2)
                        if PACER_EVERY and tq4 > 0 and slot % PACER_EVERY == 0:
                            pacer(1)
                while workq:      # quarter boundary: flush queued pieces
                    pump(100)

            # ---- drain ----------------------------------------------------
            while pvq:
                pop_pv()
            epilogue(st["gkey"], st["accs"])
            while sideq:
                sideq.pop(0)()
            for jo in range(8):
                wo_piece(NQ - 1, jo)
    nc.finalize()
    return nc


def _host_inputs(x, attn_mask, Wq, Wk, Wv, Wo):
    x = np.asarray(x, dtype=np.float32)
    attn_mask = np.asarray(attn_mask)
    Wq = np.asarray(Wq, dtype=np.float32)
    Wk = np.asarray(Wk, dtype=np.float32)
    Wv = np.asarray(Wv, dtype=np.float32)
    Wo = np.asarray(Wo, dtype=np.float32)

    cos, sin, scale = _rot_tables_np()
    cosT, sinT, scaleT = cos.T, sin.T, scale.T            # [D, T]
    # sign-fold for the partition-swap rotate-half: even d rows get -sin
    sgn = np.where(np.arange(D) % 2 == 0, -1.0, 1.0).astype(np.float32)[:, None]
    qcos = np.ascontiguousarray(np.tile(cosT * scaleT, (2, 1)), dtype=np.float32)
    qsin = np.ascontiguousarray(np.tile(sinT * scaleT * sgn, (2, 1)),
                                dtype=np.float32)
    kcos = np.ascontiguousarray(np.tile(cosT / scaleT, (2, 1)), dtype=np.float32)
    ksin = np.ascontiguousarray(np.tile(sinT / scaleT * sgn, (2, 1)),
                                dtype=np.float32)
    ident = np.eye(128, dtype=ml_dtypes.bfloat16)

    in_maps = []
    for b in range(B):
        xTb = np.ascontiguousarray(x[b].T.astype(ml_dtypes.bfloat16))
        mB16 = np.ascontiguousarray(
            (attn_mask[b, 0].T != 0).astype(ml_dtypes.bfloat16))  # {0, 1}
        for g in range(G):
            sl = slice(CG * g, CG * (g + 1))
            Wq_g, Wk_g, Wv_g = Wq[sl], Wk[sl], Wv[sl]
            in_maps.append({
                "xT": xTb,
                "maskB": mB16,
                "wq": np.ascontiguousarray(Wq_g.T.astype(ml_dtypes.bfloat16)),
                "wk": np.ascontiguousarray(Wk_g.T.astype(ml_dtypes.bfloat16)),
                "wv": np.ascontiguousarray(Wv_g.T.astype(ml_dtypes.bfloat16)),
                "wo": np.ascontiguousarray(Wo[:, sl].T.astype(ml_dtypes.bfloat16)),
                "qcos": qcos, "qsin": qsin, "kcos": kcos, "ksin": ksin,
                "ident": ident,
            })
    return in_maps


def kernel(x, attn_mask, Wq, Wk, Wv, Wo):
    from concourse.bass_utils import run_bass_kernel_spmd

    if "nc" not in _CACHE:
        _CACHE["nc"] = _build_bass()
    nc = _CACHE["nc"]

    in_maps = _host_inputs(x, attn_mask, Wq, Wk, Wv, Wo)
    res = run_bass_kernel_spmd(nc, in_maps, core_ids=list(range(NCORES)))
    _CACHE["last_results"] = res

    y = np.empty((B, T, C), dtype=np.float32)
    for b in range(B):
        acc = np.asarray(res.results[2 * b]["yT"], dtype=np.float32) + \
              np.asarray(res.results[2 * b + 1]["yT"], dtype=np.float32)
        y[b] = acc.T
    return y



# revision 116
# speedup vs baseline: 1.0001x; 1.0001x over previous
"""Trainium2 Bass kernel for nn_AttentionLayer (B=4, T=2048, C=1024, H=16, D=64).

Sharding: 8 cores = 4 batches x 2 head-groups (8 heads each). Each core
computes a partial y[b] = out_g @ Wo_g^T; host sums the two group partials
per batch and transposes back.

Single fused pipeline engineered around the per-engine floors: ScalarE exp
(256 x ~1.04us), PE matmul out-element cost (scores+PV+projections ~278us),
and DVE (mask multiplies + rotary + copies ~263us). Design:
  - bf16 throughout; k/q projected per 512-column chunk with xpos rotary
    fused on DVE (+GPSIMD for one multiply), stored as bf16 kf (full T) and
    per-quarter double-buffered qf.
  - scoresT[tk,tq] per head-pair in a double-buffered PSUM pair; exp on
    ScalarE (constant -40 bias, 1/32 scale - no row-max pass needed).
  - mask applied POST-exp as a bf16 {0,1} multiply on DVE (2x mode, mask
    broadcast over the head pair). Its emission is deferred until the pv
    pop SKEW slots later, so a TT stalled on a mask DMA never wedges the
    in-order DVE queue ahead of rotary work, and mask loads get ~20us of
    deadline slack. (GPSIMD cannot touch PSUM on real HW, so all PSUM
    reads - copies, normalize - stay on DVE/ACT.)
  - transposed PV: out[q,0:66] += pr[:,e,qtile].T @ vaug[tk,h,0:66] (cols
    64:66 ones = softmax denominator); 16 accumulators (qt-major: s=qt*2+e)
    packed in 3 PSUM banks with per-bank start/stop flags. Epilogue: one
    reciprocal + ONE broadcast tensor_tensor normalizes 7 accumulators at
    a stroke; PE transposes read adjacent (e0,e1) pairs; Wo per 128-row
    output block, output staged through a 4-deep ysb ring.
  - DMA discipline: every dma_start pays ~625ns on the single HWDGE
    descriptor-gen device and transfers drain one DMA_ENGINES queue in
    issue order, so the prologue issues exactly in dependency order
    (x0/wk interleaved 2-chunk pieces, ktabs0, wq, qtabs0, x1, ktabs1,
    x2, ktabs2, wv, x3, ktabs3, masks, ident, wo) and bulk loads never
    preempt the first-scores path.
  - quarter-0 carries all k/v projections (every later quarter re-sweeps
    all tk): injected into attention slots sequenced by DMA arrival, with
    k-chunk generators force-drained 3 slots before first use to hide the
    8-MM chain + rotary latency. The pv backlog tapers near quarter ends
    so deferred work doesn't dump into the next quarter's first slots.
  - warm-up pacer matmuls ride out the PE p-state ramp (post-idle matmuls
    run at half clock for ~3us) during the initial weight DMA.
"""

import numpy as np
import ml_dtypes

B, T, C, H, D = 4, 2048, 1024, 16, 64
G = 2                 # head groups (tensor parallel)
NCORES = B * G
CG = C // G           # 512 channels per group
JT = CG // 128        # 4 head-pairs per core
CCH = C // 128        # 8 contraction chunks
TT = T // 128         # 16 tk-tiles
NQ = 4                # tq quarters of 512
THETA = 10000.0
SCALE_BASE = 512.0

# scheduling knobs
SKEW = 19             # pv matmuls trail the scores/exp/mask stream (slots)
PR_BUFS = 23
WARMUP_PACERS = 26
PACER_EVERY = 0       # if >0: one pacer MM every N slots in quarters 1-3

_CACHE = {}


def _rot_tables_np():
    inv_freq = 1.0 / (THETA ** (np.arange(0, D, 2, dtype=np.float32) / D))
    seq = np.arange(T, dtype=np.float32)
    freqs = seq[:, None] * inv_freq[None, :]
    freqs = np.repeat(freqs, 2, axis=-1)                    # [T, D]
    base = (np.arange(0, D, 2, dtype=np.float32) + 0.4 * D) / (1.4 * D)
    power = (seq - T // 2) / SCALE_BASE
    scale = base[None, :] ** power[:, None]
    scale = np.repeat(scale, 2, axis=-1)                    # [T, D]
    return np.cos(freqs), np.sin(freqs), scale.astype(np.float32)


def _build_bass():
    import concourse.bass as bass
    import concourse.bacc as bacc
    import concourse.mybir as mybir
    import concourse.tile as tile
    from concourse.bass import ts, ds

    fp32 = mybir.dt.float32
    bf16 = mybir.dt.bfloat16
    MUL = mybir.AluOpType.mult
    ADD = mybir.AluOpType.add
    EXP = mybir.ActivationFunctionType.Exp

    nc = bacc.Bacc(None)

    xT = nc.dram_tensor("xT", [C, T], bf16, kind="ExternalInput")
    wq = nc.dram_tensor("wq", [C, CG], bf16, kind="ExternalInput")
    wk = nc.dram_tensor("wk", [C, CG], bf16, kind="ExternalInput")
    wv = nc.dram_tensor("wv", [C, CG], bf16, kind="ExternalInput")
    wo = nc.dram_tensor("wo", [CG, C], bf16, kind="ExternalInput")
    qcos = nc.dram_tensor("qcos", [128, T], fp32, kind="ExternalInput")
    qsin = nc.dram_tensor("qsin", [128, T], fp32, kind="ExternalInput")
    kcos = nc.dram_tensor("kcos", [128, T], fp32, kind="ExternalInput")
    ksin = nc.dram_tensor("ksin", [128, T], fp32, kind="ExternalInput")
    maskB = nc.dram_tensor("maskB", [T, T], bf16, kind="ExternalInput")
    ident_d = nc.dram_tensor("ident", [128, 128], bf16, kind="ExternalInput")
    yT = nc.dram_tensor("yT", [C, T], fp32, kind="ExternalOutput")

    xT_r = xT.rearrange("(cc p) t -> p cc t", p=128)        # [128, 8, T]
    maskB_r = maskB.rearrange("(tk p) q -> p tk q", p=128)  # [128, 16, T]
    SWAPM = [i + 1 - 2 * (i % 2) for i in range(32)]

    # pv accumulator group = one head-pair (jt): 8 accumulators [128, 66]
    # in 2 PSUM banks (pvA rows 0..6, pvB row 0 = s7). The accumulate
    # zero-region is a whole bank, so only the chronologically-first MM per
    # bank starts the group and the last stops it.
    PV_START = {0, 7}
    PV_STOP = {5, 7}
    MASK_ON_GPSIMD = set()  # (quarter, slot) pairs: mask-mult on Pool

    with tile.TileContext(nc) as tc:
        with (
            tc.tile_pool(name="persist", bufs=1) as persist,
            tc.tile_pool(name="xp", bufs=4) as xp,
            tc.tile_pool(name="ktabp", bufs=4) as ktabp,
            tc.tile_pool(name="qtabp", bufs=1) as qtabp,
            tc.tile_pool(name="maskp", bufs=3) as maskp,
            tc.tile_pool(name="prp", bufs=PR_BUFS) as prp,
            tc.tile_pool(name="rotp", bufs=3) as rotp,
            tc.tile_pool(name="stgp", bufs=2) as stgp,
            tc.tile_pool(name="recp", bufs=6) as recp,
            tc.tile_pool(name="ysbp", bufs=4) as ysbp,
            tc.tile_pool(name="scp", bufs=2, space="PSUM") as scp,
            tc.tile_pool(name="pvp", bufs=2, space="PSUM") as pvp,
            tc.tile_pool(name="tailp", bufs=2, space="PSUM") as tailp,
        ):
            kf = persist.tile([128, JT, T], bf16, tag="kf")
            qfq = persist.tile([128, 2, JT, 512], bf16, tag="qfq")
            vaug = persist.tile([128, TT, 8, 66], bf16, tag="vaug")
            wq_sb = persist.tile([128, CCH, CG], bf16, tag="wq")
            # wk shares the mask ring: k projections finish in quarter 0,
            # after which its 8KB slot recycles into mask buffers.
            wk_sb = maskp.tile([128, CCH, CG], bf16, tag="mask", name="wk_sb")
            wv_sb = persist.tile([128, CCH, CG], bf16, tag="wv")
            wo_sb = persist.tile([128, JT, C], bf16, tag="wo")
            oth = persist.tile([128, JT, 512], bf16, tag="oth")
            ident = persist.tile([128, 128], bf16, tag="ident")
            bias_m40 = persist.tile([128, 1], fp32, tag="biasm40")
            dmy_a = persist.tile([128, 128], bf16, tag="dmya")
            dmy_b = persist.tile([128, 256], bf16, tag="dmyb")
            dmy_s = persist.tile([128, 1], fp32, tag="dmys")

            # ---- t=0 warm-up: pacers ride out the p-state ramp + DMA head;
            # a dummy activation pulls the exp table load forward.
            nc.vector.memset(dmy_a[:], 0.0)
            nc.vector.memset(dmy_b[:], 0.0)
            nc.vector.memset(bias_m40[:], -40.0)
            nc.vector.memset(vaug[:, :, :, 64:66], 1.0)
            nc.scalar.activation(dmy_s[:], bias_m40[:], EXP,
                                 bias=bias_m40[:, :], scale=0.0)
            pacer_ps = tailp.tile([128, 512], fp32, tag="tail")
            for _ in range(WARMUP_PACERS):
                nc.tensor.matmul(pacer_ps[:, 0:256], dmy_a[:], dmy_b[:],
                                 start=True, stop=True)

            def pacer(n):
                pps = tailp.tile([128, 512], fp32, tag="tail")
                for _ in range(n):
                    nc.tensor.matmul(pps[:, 0:256], dmy_a[:], dmy_b[:],
                                     start=True, stop=True)

            # ---- weight DMA (k path first - it gates the first scores) ---
            # Two half-granularity DMAs per weight: each dma_start pays
            # ~625ns on the single HWDGE descriptor-gen device, so fewer,
            # bigger transfers win; halves keep the prelude MM chain
            # streaming. Bulk loads go through gpsimd (SWDGE) which has its
            # own desc-gen path and leaves HWDGE free for the urgent loads.
            def wdma(w_sb, w_dr, n=2):
                r = w_dr.rearrange("(cc p) j -> p cc j", p=128)
                step = CCH // n
                for i in range(0, CCH, step):
                    nc.sync.dma_start(out=w_sb[:, i:i + step, :],
                                      in_=r[:, i:i + step, :])

            def load_x(tcx):
                tsl = ds(tcx * 512, 512)
                xtc = xp.tile([128, CCH, 512], bf16, tag="x")
                nc.sync.dma_start(out=xtc[:], in_=xT_r[:, :, tsl])
                return xtc

            def load_tabs(tcx, which, eng=None):
                tsl = ds(tcx * 512, 512)
                srcs = {"q": (("tqc", qcos), ("tqs", qsin)),
                        "k": (("tkc", kcos), ("tks", ksin))}[which]
                pool = qtabp if which == "q" else ktabp
                eng = eng or nc.sync
                tabs = []
                for nm, dr in srcs:
                    t = pool.tile([128, 512], fp32, tag=nm)
                    eng.dma_start(out=t[:], in_=dr[:, tsl])
                    tabs.append(t)
                return tabs

            def rotary(ps, tabs, dst, eng2=None):
                # dst = ps*cos + pairswap(ps)*sin   (sign folded into sin)
                swp = rotp.tile([128, 512], fp32, tag="rt")
                nc.vector.stream_shuffle(swp[:], ps[:], SWAPM)
                t1 = rotp.tile([128, 512], fp32, tag="rt")
                nc.vector.tensor_tensor(t1[:], ps[:], tabs[0][:], MUL)
                t2 = rotp.tile([128, 512], fp32, tag="rt")
                (eng2 or nc.gpsimd).tensor_tensor(t2[:], swp[:], tabs[1][:], MUL)
                nc.vector.tensor_tensor(dst, t1[:], t2[:], ADD)

            # ---- prelude: x resident (all 4 chunks), k0/q0 for jt0 only;
            # everything else is injected into attention slots.
            held = {}

            def load_mask(q, half, eng=None):
                m = maskp.tile([128, 8, 512], bf16, tag="mask")
                (eng or nc.sync).dma_start(
                    out=m[:], in_=maskB_r[:, half * 8:half * 8 + 8,
                                          ds(q * 512, 512)])
                held[("mk", q, half)] = m

            # All prologue DMAs ride the sync HWDGE queue; transfers drain
            # the single DMA_ENGINES device in exactly this order. Sequence
            # by downstream need: first-scores path (x0,wk,ktabs0,qtabs0),
            # then mask half 0 (unblocks the slot-0 mask multiply on DVE),
            # x1 + wq, wv (v-tile pieces get pumped from slot ~1), k-tabs
            # and x chunks interleaved by deadline, mask half 1 before jt0's
            # tkt-8 multiply, wo/ident last.
            # interleave x0/wk piece DMAs so the k0 chain streams cc-by-cc
            xtc0 = xp.tile([128, CCH, 512], bf16, tag="x", name="xtc0")
            wk_r = wk.rearrange("(cc p) j -> p cc j", p=128)
            for i in range(0, CCH, 2):
                nc.sync.dma_start(out=xtc0[:, i:i + 2, :],
                                  in_=xT_r[:, i:i + 2, ds(0, 512)])
                nc.sync.dma_start(out=wk_sb[:, i:i + 2, :],
                                  in_=wk_r[:, i:i + 2, :])
            xall = [xtc0]
            ktabs = {0: load_tabs(0, "k")}
            wdma(wq_sb, wq, n=4)
            qtabs0 = load_tabs(0, "q")
            xall += [load_x(1)]
            ktabs[1] = load_tabs(1, "k")
            xall += [load_x(2)]
            ktabs[2] = load_tabs(2, "k")
            nc.sync.dma_start(
                out=wv_sb[:], in_=wv.rearrange("(cc p) j -> p cc j", p=128))
            xall += [load_x(3)]
            ktabs[3] = load_tabs(3, "k")
            load_mask(0, 0)
            load_mask(0, 1)
            nc.sync.dma_start(out=ident[:], in_=ident_d[:])
            nc.sync.dma_start(
                out=wo_sb[:], in_=wo.rearrange("(cc p) j -> p cc j", p=128))

            def proj_gen(w_sb, tcx, jt, fini):
                # generator piece: ~2 matmuls per step so injected work
                # never blocks the in-order PE stream for long
                ps = tailp.tile([128, 512], fp32, tag="tail")
                for h in range(4):
                    for cc in (2 * h, 2 * h + 1):
                        nc.tensor.matmul(ps[:], w_sb[:, cc, ts(jt, 128)],
                                         xall[tcx][:, cc, :],
                                         start=(cc == 0),
                                         stop=(cc == CCH - 1))
                    yield
                fini(ps)

            def kproj_gen(tcx, jt, eng2=None):
                return proj_gen(
                    wk_sb, tcx, jt,
                    lambda ps: rotary(ps[:], ktabs[tcx],
                                      kf[:, jt, ds(tcx * 512, 512)], eng2))

            def qproj_gen(tcx, tabs, jt, eng2=None):
                def fini(ps):
                    t = tabs["qt"] if isinstance(tabs, dict) else tabs
                    rotary(ps[:], t, qfq[:, tcx % 2, jt, :], eng2)
                return proj_gen(wq_sb, tcx, jt, fini)

            def kproj(tcx, jt, eng2=None):
                for _ in kproj_gen(tcx, jt, eng2):
                    pass

            def qproj(tcx, tabs, jt, eng2=None):
                for _ in qproj_gen(tcx, tabs, jt, eng2):
                    pass

            # jt0's chunk-0 projections in the prologue (k chain first: wk+x0
            # land before wq); jt1-3's ride early quarter-0 slots, which have
            # PE slack until PV kicks in at slot SKEW. Pacers bridge the PE
            # gap between the k chain and wq's arrival so the p-state ramp
            # never resets.
            kproj(0, 0, eng2=nc.vector)
            # the wq transfer is still in flight here; fill the window with
            # jt1-3's chunk-0 k chains instead of dummy pacers (their
            # rotaries also hide the first-scores wait on wq/qtabs0).
            kproj(0, 1, eng2=nc.vector)
            kproj(0, 2, eng2=nc.vector)
            kproj(0, 3, eng2=nc.vector)
            qproj(0, qtabs0, 0, eng2=nc.vector)

            # ---- injected side-work pieces -------------------------------
            def vtile_gen(tti_global):
                xt = xall[tti_global // 4]
                tsl = ts(tti_global % 4, 128)
                ps = tailp.tile([128, 8, 64], fp32, tag="tail")
                for h in range(4):
                    for cc in (2 * h, 2 * h + 1):
                        nc.tensor.matmul(ps[:, :, :], xt[:, cc, tsl],
                                         wv_sb[:, cc, :],
                                         start=(cc == 0),
                                         stop=(cc == CCH - 1))
                    yield
                nc.scalar.copy(vaug[:, tti_global, :, 0:64], ps[:, :, :])

            # ---- attention machinery -------------------------------------
            st = {"accs": None, "gkey": None, "y_written": 0}
            sideq = []                 # deferred small thunks (epilogue, wo)

            def acc_slice(accs, s):
                if s < 7:
                    return accs[0][:, s, :]
                return accs[1][:, s - 7, :]

            def epilogue(gkey, accs):
                tq4, jt = gkey
                rA = recp.tile([128, 8], fp32, tag="rec")
                nc.vector.reciprocal(rA[:, 0:7, None], accs[0][:, 0:7, 64:65])
                rB = recp.tile([128, 8], fp32, tag="rec")
                nc.vector.reciprocal(rB[:, 0:1, None], accs[1][:, 0:1, 64:65])

                # normalize all 7 pvA accumulators in ONE tensor_tensor with
                # the reciprocals broadcast along d (vs 7 serial TSPs); the
                # transposes then gather (s=qt, s=qt+4) as a strided slice.
                stg = stgp.tile([128, 8, 64], bf16, tag="stg")
                nc.vector.tensor_tensor(
                    stg[:, 0:7, :], accs[0][:, 0:7, 0:64],
                    rA[:, 0:7, None].broadcast_to((128, 7, 64)), MUL)
                nc.vector.tensor_scalar_mul(
                    stg[:, 7, :], accs[1][:, 0, 0:64], rB[:, 0:1])

                tb = tailp.tile([128, 4, 128], bf16, tag="tail")
                for qt in range(4):
                    nc.tensor.transpose(
                        tb[:, qt, :],
                        stg[:, 2 * qt:2 * qt + 2, :], ident[:])
                    nc.vector.tensor_copy(
                        oth[:, jt, ds(qt * 128, 128)], tb[:, qt, :])

            def wo_piece(tq4, jo):
                py = tailp.tile([128, 512], fp32, tag="tail")
                for cc in range(JT):
                    nc.tensor.matmul(py[:], wo_sb[:, cc, ts(jo, 128)],
                                     oth[:, cc, :],
                                     start=(cc == 0), stop=(cc == JT - 1))
                ysb = ysbp.tile([128, 512], fp32, tag="ysb")
                nc.vector.tensor_copy(ysb[:], py[:])
                nc.sync.dma_start(out=yT[ts(jo, 128), ds(tq4 * 512, 512)],
                                  in_=ysb[:])

            def ensure_group(gkey):
                if st["gkey"] == gkey:
                    return
                if st["gkey"] is not None:
                    epilogue(st["gkey"], st["accs"])
                    fin_tq4, fin_jt = st["gkey"]
                    if fin_jt == JT - 1:   # quarter's oth complete -> Wo
                        for jo in range(8):
                            sideq.append(
                                lambda tq4=fin_tq4, jo=jo: wo_piece(tq4, jo))
                pvA = pvp.tile([128, 7, 66], fp32, tag="pv")
                pvB = pvp.tile([128, 1, 66], fp32, tag="pv")
                st["accs"] = (pvA, pvB)
                st["gkey"] = gkey

            def emit_pv(pr, jt, tkt, gkey):
                ensure_group(gkey)
                for e in range(2):
                    h = jt * 2 + e
                    for qt in range(4):
                        s = qt * 2 + e
                        nc.tensor.matmul(
                            acc_slice(st["accs"], s),
                            pr[:, e, ts(qt, 128)],
                            vaug[:, tkt, h, 0:66],
                            start=(tkt == 0 and s in PV_START),
                            stop=(tkt == TT - 1 and s in PV_STOP),
                            skip_group_check=True)

            pvq = []                  # [(pr, jt, tkt, gkey, mb, r, slot), ...]
            workq = []                # [(key, generator)] fine-grained pieces

            def pop_pv():
                # DVE mask multiplies are emitted HERE, SKEW slots after the
                # exp: rotaries emitted near the exp slot never queue on DVE
                # behind a TT stalled on a mask DMA, and mask loads gain
                # ~20us of deadline slack. Pool-offloaded multiplies were
                # already emitted at exp time (their latency hides in the
                # skew and they never block DVE).
                pr, jt, tkt, gkey, mb, r, slot = pvq.pop(0)
                force(("v", tkt))           # vaug ready for pv
                if (gkey[0], slot) not in MASK_ON_GPSIMD:
                    nc.vector.tensor_tensor(
                        pr[:, :, :], pr[:, :, :],
                        mb[:, r, None, :].broadcast_to((128, 2, 512)), MUL)
                emit_pv(pr, jt, tkt, gkey)

            def pump(nsteps):
                while nsteps > 0 and workq:
                    key, g = workq[0]
                    try:
                        next(g)
                        nsteps -= 1
                    except StopIteration:
                        workq.pop(0)

            def force(key):
                # emission-order deadline: finish FIFO head pieces up to and
                # including `key` so dependent reads are emitted after writes
                while any(k == key for k, _ in workq):
                    k0, g = workq[0]
                    try:
                        next(g)
                    except StopIteration:
                        workq.pop(0)

            # ---- main loop ------------------------------------------------

            for tq4 in range(NQ):
                qsl = ds(tq4 * 512, 512)
                if tq4 > 0:
                    load_mask(tq4, 1)   # first half was prefetched

                inject = {}

                def add(slot, fn):
                    inject.setdefault(slot, []).append(fn)

                def addg(slot, key, mk):
                    # queue a generator piece at slot; tracked for deadlines
                    def starter():
                        workq.append((key, mk()))
                    add(slot, starter)

                def addk(slot, tcx, jt):
                    addg(slot, ("k", tcx, jt), lambda: kproj_gen(tcx, jt))

                def addq(slot, tcx, jt, tabs):
                    addg(slot, ("q", tcx, jt),
                         lambda: qproj_gen(tcx, tabs, jt))

                def addv(slot, i):
                    addg(slot, ("v", i), lambda: vtile_gen(i))

                if tq4 == 0:
                    # remaining k chunks per jt (deadline slot jt*16+4c),
                    # k0/q0 projections for jt 1-3 (deadline jt*16), all 16
                    # v tiles (deadline tt+SKEW), q quarter 1 late.
                    # workq is FIFO and pump pops the head only, so adds are
                    # sequenced by DMA arrival: x0/wq-based first, then the
                    # x1/x2/x3-gated chunks as close to their force slot as
                    # possible so a not-yet-landed DMA never wedges the head.
                    addk(0, 1, 0)                    # x1-based
                    addq(3, 0, 1, qtabs0)
                    addk(4, 2, 0)                    # x2-based
                    addq(7, 0, 2, qtabs0)
                    addv(7, 0)                       # wv-gated
                    addk(9, 3, 0)                    # x3-based
                    addv(9, 1)
                    addq(11, 0, 3, qtabs0)
                    addv(12, 2)
                    addv(13, 3)
                    addk(14, 1, 1)
                    addv(15, 4)
                    addk(16, 2, 1)
                    addv(17, 5)
                    addk(18, 3, 1)
                    addv(19, 6)
                    addv(20, 7)
                    addv(22, 8)                      # v8-11 (x2-based)
                    addv(23, 9)
                    addv(24, 10)
                    addv(25, 11)
                    addv(26, 12)                     # v12-15 (x3-based)
                    addv(27, 13)
                    addv(28, 14)
                    addv(29, 15)
                    addk(31, 1, 2)
                    addk(34, 2, 2)
                    addk(37, 3, 2)
                    addk(44, 1, 3)
                    addk(46, 2, 3)
                    addk(48, 3, 3)
                    add(50, lambda: held.__setitem__("qt", load_tabs(1, "q")))
                    for jt in range(JT):
                        addq(52 + 3 * jt, 1, jt, held)
                elif tq4 < NQ - 1:
                    add(24, lambda tq4=tq4: held.__setitem__(
                        "qt", load_tabs(tq4 + 1, "q")))
                    for jt in range(JT):
                        addq(28 + 3 * jt, tq4 + 1, jt, held)
                if tq4 < NQ - 1:
                    add(44, lambda tq4=tq4: load_mask(tq4 + 1, 0))

                for jt in range(JT):
                    for tkt in range(TT):
                        slot = jt * 16 + tkt
                        if tkt == 0:
                            force(("q", tq4, jt))   # qfq ready for scores
                        if tkt % 4 == 0:
                            force(("k", tkt // 4, jt))  # kf ready for scores
                        elif tkt % 4 == 1 and tkt < 13:
                            # drain the NEXT chunk 3 slots early so its
                            # 8-MM chain + rotary latency is hidden
                            force(("k", tkt // 4 + 1, jt))
                        ps = scp.tile([128, 2, 512], fp32, tag="sc")
                        mb = held[("mk", tq4, tkt // 8)]
                        r = tkt % 8
                        for e in range(2):
                            nc.tensor.matmul(
                                ps[:, e, :],
                                kf[ds(e * 64, 64), jt, ts(tkt, 128)],
                                qfq[ds(e * 64, 64), tq4 % 2, jt, :],
                                start=True, stop=True)
                        pump(2)
                        if tq4 == NQ - 1 and slot >= 36:
                            target = 1 if slot >= 52 else 4
                        elif slot >= 56:
                            target = 8   # taper into the quarter boundary
                        else:
                            target = SKEW
                        while len(pvq) >= target:
                            pop_pv()
                        pump(2)
                        pr = prp.tile([128, 2, 512], bf16, tag="pr")
                        nc.scalar.activation(pr[:, :, :], ps[:, :, :],
                                             EXP, bias=bias_m40[:, :],
                                             scale=0.03125)
                        if (tq4, slot) in MASK_ON_GPSIMD:
                            nc.gpsimd.tensor_tensor(
                                pr[:, :, :], pr[:, :, :],
                                mb[:, r, None, :].broadcast_to((128, 2, 512)),
                                MUL)
                        pvq.append((pr, jt, tkt, (tq4, jt), mb, r, slot))
                        for _ in range(2 if tq4 == NQ - 1 else 1):
                            if sideq:
                                sideq.pop(0)()
                        for fn in inject.get(slot, ()):
                            fn()
                        pump(2)
                        if PACER_EVERY and tq4 > 0 and slot % PACER_EVERY == 0:
                            pacer(1)
                while workq:      # quarter boundary: flush queued pieces
                    pump(100)

            # ---- drain ----------------------------------------------------
            while pvq:
                pop_pv()
            epilogue(st["gkey"], st["accs"])
            while sideq:
                sideq.pop(0)()
            for jo in range(8):
                wo_piece(NQ - 1, jo)
    nc.finalize()
    return nc


def _host_inputs(x, attn_mask, Wq, Wk, Wv, Wo):
    x = np.asarray(x, dtype=np.float32)
    attn_mask = np.asarray(attn_mask)
    Wq = np.asarray(Wq, dtype=np.float32)
    Wk = np.asarray(Wk, dtype=np.float32)
    Wv = np.asarray(Wv, dtype=np.float32)
    Wo = np.asarray(Wo, dtype=np.float32)

    cos, sin, scale = _rot_tables_np()
    cosT, sinT, scaleT = cos.T, sin.T, scale.T            # [D, T]
    # sign-fold for the partition-swap rotate-half: even d rows get -sin
    sgn = np.where(np.arange(D) % 2 == 0, -1.0, 1.0).astype(np.float32)[:, None]
    qcos = np.ascontiguousarray(np.tile(cosT * scaleT, (2, 1)), dtype=np.float32)
    qsin = np.ascontiguousarray(np.tile(sinT * scaleT * sgn, (2, 1)),
                                dtype=np.float32)
    kcos = np.ascontiguousarray(np.tile(cosT / scaleT, (2, 1)), dtype=np.float32)
    ksin = np.ascontiguousarray(np.tile(sinT / scaleT * sgn, (2, 1)),
                                dtype=np.float32)
    ident = np.eye(128, dtype=ml_dtypes.bfloat16)

    in_maps = []
    for b in range(B):
        xTb = np.ascontiguousarray(x[b].T.astype(ml_dtypes.bfloat16))
        mB16 = np.ascontiguousarray(
            (attn_mask[b, 0].T != 0).astype(ml_dtypes.bfloat16))  # {0, 1}
        for g in range(G):
            sl = slice(CG * g, CG * (g + 1))
            Wq_g, Wk_g, Wv_g = Wq[sl], Wk[sl], Wv[sl]
            in_maps.append({
                "xT": xTb,
                "maskB": mB16,
                "wq": np.ascontiguousarray(Wq_g.T.astype(ml_dtypes.bfloat16)),
                "wk": np.ascontiguousarray(Wk_g.T.astype(ml_dtypes.bfloat16)),
                "wv": np.ascontiguousarray(Wv_g.T.astype(ml_dtypes.bfloat16)),
                "wo": np.ascontiguousarray(Wo[:, sl].T.astype(ml_dtypes.bfloat16)),
                "qcos": qcos, "qsin": qsin, "kcos": kcos, "ksin": ksin,
                "ident": ident,
            })
    return in_maps


def kernel(x, attn_mask, Wq, Wk, Wv, Wo):
    from concourse.bass_utils import run_bass_kernel_spmd

    if "nc" not in _CACHE:
        _CACHE["nc"] = _build_bass()
    nc = _CACHE["nc"]

    in_maps = _host_inputs(x, attn_mask, Wq, Wk, Wv, Wo)
    res = run_bass_kernel_spmd(nc, in_maps, core_ids=list(range(NCORES)))
    _CACHE["last_results"] = res

    y = np.empty((B, T, C), dtype=np.float32)
    for b in range(B):
        acc = np.asarray(res.results[2 * b]["yT"], dtype=np.float32) + \
              np.asarray(res.results[2 * b + 1]["yT"], dtype=np.float32)
        y[b] = acc.T
    return y



# revision 123
# speedup vs baseline: 1.0022x; 1.0021x over previous
"""Trainium2 Bass kernel for nn_AttentionLayer (B=4, T=2048, C=1024, H=16, D=64).

Sharding: 8 cores = 4 batches x 2 head-groups (8 heads each). Each core
computes a partial y[b] = out_g @ Wo_g^T; host sums the two group partials
per batch and transposes back.

Single fused pipeline engineered around the per-engine floors: ScalarE exp
(256 x ~1.04us), PE matmul out-element cost (scores+PV+projections ~278us),
and DVE (mask multiplies + rotary + copies ~263us). Design:
  - bf16 throughout; k/q projected per 512-column chunk with xpos rotary
    fused on DVE (+GPSIMD for one multiply), stored as bf16 kf (full T) and
    per-quarter double-buffered qf.
  - scoresT[tk,tq] per head-pair in a double-buffered PSUM pair; exp on
    ScalarE (constant -40 bias, 1/32 scale - no row-max pass needed).
  - mask applied POST-exp as a bf16 {0,1} multiply on DVE (2x mode, mask
    broadcast over the head pair). Its emission is deferred until the pv
    pop SKEW slots later, so a TT stalled on a mask DMA never wedges the
    in-order DVE queue ahead of rotary work, and mask loads get ~20us of
    deadline slack. (GPSIMD cannot touch PSUM on real HW, so all PSUM
    reads - copies, normalize - stay on DVE/ACT.)
  - transposed PV: out[q,0:66] += pr[:,e,qtile].T @ vaug[tk,h,0:66] (cols
    64:66 ones = softmax denominator); 16 accumulators (qt-major: s=qt*2+e)
    packed in 3 PSUM banks with per-bank start/stop flags. Epilogue: one
    reciprocal + ONE broadcast tensor_tensor normalizes 7 accumulators at
    a stroke; PE transposes read adjacent (e0,e1) pairs; Wo per 128-row
    output block, output staged through a 4-deep ysb ring.
  - DMA discipline: every dma_start pays ~625ns on the single HWDGE
    descriptor-gen device and transfers drain one DMA_ENGINES queue in
    issue order, so the prologue issues exactly in dependency order
    (x0/wk interleaved 2-chunk pieces, ktabs0, wq, qtabs0, x1, ktabs1,
    x2, ktabs2, wv, x3, ktabs3, masks, ident, wo) and bulk loads never
    preempt the first-scores path.
  - quarter-0 carries all k/v projections (every later quarter re-sweeps
    all tk): injected into attention slots sequenced by DMA arrival, with
    k-chunk generators force-drained 3 slots before first use to hide the
    8-MM chain + rotary latency. The pv backlog tapers near quarter ends
    so deferred work doesn't dump into the next quarter's first slots.
  - warm-up pacer matmuls ride out the PE p-state ramp (post-idle matmuls
    run at half clock for ~3us) during the initial weight DMA.
"""

import numpy as np
import ml_dtypes

B, T, C, H, D = 4, 2048, 1024, 16, 64
G = 2                 # head groups (tensor parallel)
NCORES = B * G
CG = C // G           # 512 channels per group
JT = CG // 128        # 4 head-pairs per core
CCH = C // 128        # 8 contraction chunks
TT = T // 128         # 16 tk-tiles
NQ = 4                # tq quarters of 512
THETA = 10000.0
SCALE_BASE = 512.0

# scheduling knobs
SKEW = 19             # pv matmuls trail the scores/exp/mask stream (slots)
PR_BUFS = 23
WARMUP_PACERS = 10
PACER_EVERY = 0       # if >0: one pacer MM every N slots in quarters 1-3

_CACHE = {}


def _rot_tables_np():
    inv_freq = 1.0 / (THETA ** (np.arange(0, D, 2, dtype=np.float32) / D))
    seq = np.arange(T, dtype=np.float32)
    freqs = seq[:, None] * inv_freq[None, :]
    freqs = np.repeat(freqs, 2, axis=-1)                    # [T, D]
    base = (np.arange(0, D, 2, dtype=np.float32) + 0.4 * D) / (1.4 * D)
    power = (seq - T // 2) / SCALE_BASE
    scale = base[None, :] ** power[:, None]
    scale = np.repeat(scale, 2, axis=-1)                    # [T, D]
    return np.cos(freqs), np.sin(freqs), scale.astype(np.float32)


def _build_bass():
    import concourse.bass as bass
    import concourse.bacc as bacc
    import concourse.mybir as mybir
    import concourse.tile as tile
    from concourse.bass import ts, ds

    fp32 = mybir.dt.float32
    bf16 = mybir.dt.bfloat16
    MUL = mybir.AluOpType.mult
    ADD = mybir.AluOpType.add
    EXP = mybir.ActivationFunctionType.Exp

    nc = bacc.Bacc(None)

    xT = nc.dram_tensor("xT", [C, T], bf16, kind="ExternalInput")
    wq = nc.dram_tensor("wq", [C, CG], bf16, kind="ExternalInput")
    wk = nc.dram_tensor("wk", [C, CG], bf16, kind="ExternalInput")
    wv = nc.dram_tensor("wv", [C, CG], bf16, kind="ExternalInput")
    wo = nc.dram_tensor("wo", [CG, C], bf16, kind="ExternalInput")
    qcos = nc.dram_tensor("qcos", [128, T], fp32, kind="ExternalInput")
    qsin = nc.dram_tensor("qsin", [128, T], fp32, kind="ExternalInput")
    kcos = nc.dram_tensor("kcos", [128, T], fp32, kind="ExternalInput")
    ksin = nc.dram_tensor("ksin", [128, T], fp32, kind="ExternalInput")
    maskB = nc.dram_tensor("maskB", [T, T], bf16, kind="ExternalInput")
    ident_d = nc.dram_tensor("ident", [128, 128], bf16, kind="ExternalInput")
    yT = nc.dram_tensor("yT", [C, T], fp32, kind="ExternalOutput")

    xT_r = xT.rearrange("(cc p) t -> p cc t", p=128)        # [128, 8, T]
    maskB_r = maskB.rearrange("(tk p) q -> p tk q", p=128)  # [128, 16, T]
    SWAPM = [i + 1 - 2 * (i % 2) for i in range(32)]

    # pv accumulator group = one head-pair (jt): 8 accumulators [128, 66]
    # in 2 PSUM banks (pvA rows 0..6, pvB row 0 = s7). The accumulate
    # zero-region is a whole bank, so only the chronologically-first MM per
    # bank starts the group and the last stops it.
    PV_START = {0, 7}
    PV_STOP = {5, 7}
    MASK_ON_GPSIMD = set()  # (quarter, slot) pairs: mask-mult on Pool

    with tile.TileContext(nc) as tc:
        with (
            tc.tile_pool(name="persist", bufs=1) as persist,
            tc.tile_pool(name="xp", bufs=4) as xp,
            tc.tile_pool(name="ktabp", bufs=4) as ktabp,
            tc.tile_pool(name="qtabp", bufs=1) as qtabp,
            tc.tile_pool(name="maskp", bufs=3) as maskp,
            tc.tile_pool(name="prp", bufs=PR_BUFS) as prp,
            tc.tile_pool(name="rotp", bufs=3) as rotp,
            tc.tile_pool(name="stgp", bufs=2) as stgp,
            tc.tile_pool(name="recp", bufs=6) as recp,
            tc.tile_pool(name="ysbp", bufs=4) as ysbp,
            tc.tile_pool(name="scp", bufs=2, space="PSUM") as scp,
            tc.tile_pool(name="pvp", bufs=2, space="PSUM") as pvp,
            tc.tile_pool(name="tailp", bufs=2, space="PSUM") as tailp,
        ):
            kf = persist.tile([128, JT, T], bf16, tag="kf")
            qfq = persist.tile([128, 2, JT, 512], bf16, tag="qfq")
            vaug = persist.tile([128, TT, 8, 66], bf16, tag="vaug")
            wq_sb = persist.tile([128, CCH, CG], bf16, tag="wq")
            # wk shares the mask ring: k projections finish in quarter 0,
            # after which its 8KB slot recycles into mask buffers.
            wk_sb = maskp.tile([128, CCH, CG], bf16, tag="mask", name="wk_sb")
            wv_sb = persist.tile([128, CCH, CG], bf16, tag="wv")
            wo_sb = persist.tile([128, JT, C], bf16, tag="wo")
            oth = persist.tile([128, JT, 512], bf16, tag="oth")
            ident = persist.tile([128, 128], bf16, tag="ident")
            bias_m40 = persist.tile([128, 1], fp32, tag="biasm40")
            dmy_a = persist.tile([128, 128], bf16, tag="dmya")
            dmy_b = persist.tile([128, 256], bf16, tag="dmyb")
            dmy_s = persist.tile([128, 1], fp32, tag="dmys")

            # ---- t=0 warm-up: pacers ride out the p-state ramp + DMA head;
            # a dummy activation pulls the exp table load forward.
            nc.vector.memset(dmy_a[:], 0.0)
            nc.vector.memset(dmy_b[:], 0.0)
            nc.vector.memset(bias_m40[:], -40.0)
            nc.vector.memset(vaug[:, :, :, 64:66], 1.0)
            nc.scalar.activation(dmy_s[:], bias_m40[:], EXP,
                                 bias=bias_m40[:, :], scale=0.0)
            pacer_ps = tailp.tile([128, 512], fp32, tag="tail")
            for _ in range(WARMUP_PACERS):
                nc.tensor.matmul(pacer_ps[:, 0:256], dmy_a[:], dmy_b[:],
                                 start=True, stop=True)

            def pacer(n):
                pps = tailp.tile([128, 512], fp32, tag="tail")
                for _ in range(n):
                    nc.tensor.matmul(pps[:, 0:256], dmy_a[:], dmy_b[:],
                                     start=True, stop=True)

            # ---- weight DMA (k path first - it gates the first scores) ---
            # Two half-granularity DMAs per weight: each dma_start pays
            # ~625ns on the single HWDGE descriptor-gen device, so fewer,
            # bigger transfers win; halves keep the prelude MM chain
            # streaming. Bulk loads go through gpsimd (SWDGE) which has its
            # own desc-gen path and leaves HWDGE free for the urgent loads.
            def wdma(w_sb, w_dr, n=2):
                r = w_dr.rearrange("(cc p) j -> p cc j", p=128)
                step = CCH // n
                for i in range(0, CCH, step):
                    nc.sync.dma_start(out=w_sb[:, i:i + step, :],
                                      in_=r[:, i:i + step, :])

            def load_x(tcx):
                tsl = ds(tcx * 512, 512)
                xtc = xp.tile([128, CCH, 512], bf16, tag="x")
                nc.sync.dma_start(out=xtc[:], in_=xT_r[:, :, tsl])
                return xtc

            def load_tabs(tcx, which, eng=None):
                tsl = ds(tcx * 512, 512)
                srcs = {"q": (("tqc", qcos), ("tqs", qsin)),
                        "k": (("tkc", kcos), ("tks", ksin))}[which]
                pool = qtabp if which == "q" else ktabp
                eng = eng or nc.sync
                tabs = []
                for nm, dr in srcs:
                    t = pool.tile([128, 512], fp32, tag=nm)
                    eng.dma_start(out=t[:], in_=dr[:, tsl])
                    tabs.append(t)
                return tabs

            def rotary(ps, tabs, dst, eng2=None):
                # dst = ps*cos + pairswap(ps)*sin   (sign folded into sin)
                swp = rotp.tile([128, 512], fp32, tag="rt")
                nc.vector.stream_shuffle(swp[:], ps[:], SWAPM)
                t1 = rotp.tile([128, 512], fp32, tag="rt")
                nc.vector.tensor_tensor(t1[:], ps[:], tabs[0][:], MUL)
                t2 = rotp.tile([128, 512], fp32, tag="rt")
                (eng2 or nc.gpsimd).tensor_tensor(t2[:], swp[:], tabs[1][:], MUL)
                nc.vector.tensor_tensor(dst, t1[:], t2[:], ADD)

            # ---- prelude: x resident (all 4 chunks), k0/q0 for jt0 only;
            # everything else is injected into attention slots.
            held = {}

            def load_mask(q, half, eng=None):
                m = maskp.tile([128, 8, 512], bf16, tag="mask")
                (eng or nc.sync).dma_start(
                    out=m[:], in_=maskB_r[:, half * 8:half * 8 + 8,
                                          ds(q * 512, 512)])
                held[("mk", q, half)] = m

            # All prologue DMAs ride the sync HWDGE queue; transfers drain
            # the single DMA_ENGINES device in exactly this order. Sequence
            # by downstream need: first-scores path (x0,wk,ktabs0,qtabs0),
            # then mask half 0 (unblocks the slot-0 mask multiply on DVE),
            # x1 + wq, wv (v-tile pieces get pumped from slot ~1), k-tabs
            # and x chunks interleaved by deadline, mask half 1 before jt0's
            # tkt-8 multiply, wo/ident last.
            # interleave x0/wk piece DMAs so the k0 chain streams cc-by-cc
            xtc0 = xp.tile([128, CCH, 512], bf16, tag="x", name="xtc0")
            wk_r = wk.rearrange("(cc p) j -> p cc j", p=128)
            for i in range(0, CCH, 2):
                nc.sync.dma_start(out=xtc0[:, i:i + 2, :],
                                  in_=xT_r[:, i:i + 2, ds(0, 512)])
                nc.sync.dma_start(out=wk_sb[:, i:i + 2, :],
                                  in_=wk_r[:, i:i + 2, :])
            xall = [xtc0]
            ktabs = {0: load_tabs(0, "k")}
            wdma(wq_sb, wq, n=4)
            qtabs0 = load_tabs(0, "q")
            xall += [load_x(1)]
            ktabs[1] = load_tabs(1, "k")
            xall += [load_x(2)]
            ktabs[2] = load_tabs(2, "k")
            nc.sync.dma_start(
                out=wv_sb[:], in_=wv.rearrange("(cc p) j -> p cc j", p=128))
            xall += [load_x(3)]
            ktabs[3] = load_tabs(3, "k")
            load_mask(0, 0)
            load_mask(0, 1)
            nc.sync.dma_start(out=ident[:], in_=ident_d[:])
            nc.sync.dma_start(
                out=wo_sb[:], in_=wo.rearrange("(cc p) j -> p cc j", p=128))

            def proj_gen(w_sb, tcx, jt, fini):
                # generator piece: ~2 matmuls per step so injected work
                # never blocks the in-order PE stream for long
                ps = tailp.tile([128, 512], fp32, tag="tail")
                for h in range(4):
                    for cc in (2 * h, 2 * h + 1):
                        nc.tensor.matmul(ps[:], w_sb[:, cc, ts(jt, 128)],
                                         xall[tcx][:, cc, :],
                                         start=(cc == 0),
                                         stop=(cc == CCH - 1))
                    yield
                fini(ps)

            def kproj_gen(tcx, jt, eng2=None):
                return proj_gen(
                    wk_sb, tcx, jt,
                    lambda ps: rotary(ps[:], ktabs[tcx],
                                      kf[:, jt, ds(tcx * 512, 512)], eng2))

            def qproj_gen(tcx, tabs, jt, eng2=None):
                def fini(ps):
                    t = tabs["qt"] if isinstance(tabs, dict) else tabs
                    rotary(ps[:], t, qfq[:, tcx % 2, jt, :], eng2)
                return proj_gen(wq_sb, tcx, jt, fini)

            def kproj(tcx, jt, eng2=None):
                for _ in kproj_gen(tcx, jt, eng2):
                    pass

            def qproj(tcx, tabs, jt, eng2=None):
                for _ in qproj_gen(tcx, tabs, jt, eng2):
                    pass

            # jt0's chunk-0 projections in the prologue (k chain first: wk+x0
            # land before wq); jt1-3's ride early quarter-0 slots, which have
            # PE slack until PV kicks in at slot SKEW. Pacers bridge the PE
            # gap between the k chain and wq's arrival so the p-state ramp
            # never resets.
            kproj(0, 0, eng2=nc.vector)
            # the wq transfer is still in flight here; fill the window with
            # jt1-3's chunk-0 k chains instead of dummy pacers (their
            # rotaries also hide the first-scores wait on wq/qtabs0).
            kproj(0, 1, eng2=nc.vector)
            kproj(0, 2, eng2=nc.vector)
            kproj(0, 3, eng2=nc.vector)
            qproj(0, qtabs0, 0, eng2=nc.vector)

            # ---- injected side-work pieces -------------------------------
            def vtile_gen(tti_global):
                xt = xall[tti_global // 4]
                tsl = ts(tti_global % 4, 128)
                ps = tailp.tile([128, 8, 64], fp32, tag="tail")
                for h in range(4):
                    for cc in (2 * h, 2 * h + 1):
                        nc.tensor.matmul(ps[:, :, :], xt[:, cc, tsl],
                                         wv_sb[:, cc, :],
                                         start=(cc == 0),
                                         stop=(cc == CCH - 1))
                    yield
                nc.scalar.copy(vaug[:, tti_global, :, 0:64], ps[:, :, :])

            # ---- attention machinery -------------------------------------
            st = {"accs": None, "gkey": None, "y_written": 0}
            sideq = []                 # deferred small thunks (epilogue, wo)

            def acc_slice(accs, s):
                if s < 7:
                    return accs[0][:, s, :]
                return accs[1][:, s - 7, :]

            def epilogue(gkey, accs):
                tq4, jt = gkey
                rA = recp.tile([128, 8], fp32, tag="rec")
                nc.vector.reciprocal(rA[:, 0:7, None], accs[0][:, 0:7, 64:65])
                rB = recp.tile([128, 8], fp32, tag="rec")
                nc.vector.reciprocal(rB[:, 0:1, None], accs[1][:, 0:1, 64:65])

                # normalize all 7 pvA accumulators in ONE tensor_tensor with
                # the reciprocals broadcast along d (vs 7 serial TSPs); the
                # transposes then gather (s=qt, s=qt+4) as a strided slice.
                stg = stgp.tile([128, 8, 64], bf16, tag="stg")
                nc.vector.tensor_tensor(
                    stg[:, 0:7, :], accs[0][:, 0:7, 0:64],
                    rA[:, 0:7, None].broadcast_to((128, 7, 64)), MUL)
                nc.vector.tensor_scalar_mul(
                    stg[:, 7, :], accs[1][:, 0, 0:64], rB[:, 0:1])

                tb = tailp.tile([128, 4, 128], bf16, tag="tail")
                for qt in range(4):
                    nc.tensor.transpose(
                        tb[:, qt, :],
                        stg[:, 2 * qt:2 * qt + 2, :], ident[:])
                    nc.vector.tensor_copy(
                        oth[:, jt, ds(qt * 128, 128)], tb[:, qt, :])

            def wo_piece(tq4, jo):
                py = tailp.tile([128, 512], fp32, tag="tail")
                for cc in range(JT):
                    nc.tensor.matmul(py[:], wo_sb[:, cc, ts(jo, 128)],
                                     oth[:, cc, :],
                                     start=(cc == 0), stop=(cc == JT - 1))
                ysb = ysbp.tile([128, 512], fp32, tag="ysb")
                nc.vector.tensor_copy(ysb[:], py[:])
                nc.sync.dma_start(out=yT[ts(jo, 128), ds(tq4 * 512, 512)],
                                  in_=ysb[:])

            def ensure_group(gkey):
                if st["gkey"] == gkey:
                    return
                if st["gkey"] is not None:
                    epilogue(st["gkey"], st["accs"])
                    fin_tq4, fin_jt = st["gkey"]
                    if fin_jt == JT - 1:   # quarter's oth complete -> Wo
                        for jo in range(8):
                            sideq.append(
                                lambda tq4=fin_tq4, jo=jo: wo_piece(tq4, jo))
                pvA = pvp.tile([128, 7, 66], fp32, tag="pv")
                pvB = pvp.tile([128, 1, 66], fp32, tag="pv")
                st["accs"] = (pvA, pvB)
                st["gkey"] = gkey

            def emit_pv(pr, jt, tkt, gkey):
                ensure_group(gkey)
                for e in range(2):
                    h = jt * 2 + e
                    for qt in range(4):
                        s = qt * 2 + e
                        nc.tensor.matmul(
                            acc_slice(st["accs"], s),
                            pr[:, e, ts(qt, 128)],
                            vaug[:, tkt, h, 0:66],
                            start=(tkt == 0 and s in PV_START),
                            stop=(tkt == TT - 1 and s in PV_STOP),
                            skip_group_check=True)

            pvq = []                  # [(pr, jt, tkt, gkey, mb, r, slot), ...]
            workq = []                # [(key, generator)] fine-grained pieces

            def pop_pv():
                # DVE mask multiplies are emitted HERE, SKEW slots after the
                # exp: rotaries emitted near the exp slot never queue on DVE
                # behind a TT stalled on a mask DMA, and mask loads gain
                # ~20us of deadline slack. Pool-offloaded multiplies were
                # already emitted at exp time (their latency hides in the
                # skew and they never block DVE).
                pr, jt, tkt, gkey, mb, r, slot = pvq.pop(0)
                force(("v", tkt))           # vaug ready for pv
                if (gkey[0], slot) not in MASK_ON_GPSIMD:
                    nc.vector.tensor_tensor(
                        pr[:, :, :], pr[:, :, :],
                        mb[:, r, None, :].broadcast_to((128, 2, 512)), MUL)
                emit_pv(pr, jt, tkt, gkey)

            def pump(nsteps):
                while nsteps > 0 and workq:
                    key, g = workq[0]
                    try:
                        next(g)
                        nsteps -= 1
                    except StopIteration:
                        workq.pop(0)

            def force(key):
                # emission-order deadline: finish FIFO head pieces up to and
                # including `key` so dependent reads are emitted after writes
                while any(k == key for k, _ in workq):
                    k0, g = workq[0]
                    try:
                        next(g)
                    except StopIteration:
                        workq.pop(0)

            # ---- main loop ------------------------------------------------

            for tq4 in range(NQ):
                qsl = ds(tq4 * 512, 512)
                if tq4 > 0:
                    load_mask(tq4, 1)   # first half was prefetched

                inject = {}

                def add(slot, fn):
                    inject.setdefault(slot, []).append(fn)

                def addg(slot, key, mk):
                    # queue a generator piece at slot; tracked for deadlines
                    def starter():
                        workq.append((key, mk()))
                    add(slot, starter)

                def addk(slot, tcx, jt):
                    addg(slot, ("k", tcx, jt), lambda: kproj_gen(tcx, jt))

                def addq(slot, tcx, jt, tabs):
                    addg(slot, ("q", tcx, jt),
                         lambda: qproj_gen(tcx, tabs, jt))

                def addv(slot, i):
                    addg(slot, ("v", i), lambda: vtile_gen(i))

                if tq4 == 0:
                    # remaining k chunks per jt (deadline slot jt*16+4c),
                    # k0/q0 projections for jt 1-3 (deadline jt*16), all 16
                    # v tiles (deadline tt+SKEW), q quarter 1 late.
                    # workq is FIFO and pump pops the head only, so adds are
                    # sequenced by DMA arrival: x0/wq-based first, then the
                    # x1/x2/x3-gated chunks as close to their force slot as
                    # possible so a not-yet-landed DMA never wedges the head.
                    addk(0, 1, 0)                    # x1-based
                    addq(3, 0, 1, qtabs0)
                    addk(4, 2, 0)                    # x2-based
                    addq(7, 0, 2, qtabs0)
                    addv(7, 0)                       # wv-gated
                    addk(9, 3, 0)                    # x3-based
                    addv(9, 1)
                    addq(11, 0, 3, qtabs0)
                    addv(12, 2)
                    addv(13, 3)
                    addk(14, 1, 1)
                    addv(15, 4)
                    addk(16, 2, 1)
                    addv(17, 5)
                    addk(18, 3, 1)
                    addv(19, 6)
                    addv(20, 7)
                    addv(22, 8)                      # v8-11 (x2-based)
                    addv(23, 9)
                    addv(24, 10)
                    addv(25, 11)
                    addv(26, 12)                     # v12-15 (x3-based)
                    addv(27, 13)
                    addv(28, 14)
                    addv(29, 15)
                    addk(31, 1, 2)
                    addk(34, 2, 2)
                    addk(37, 3, 2)
                    addk(44, 1, 3)
                    addk(46, 2, 3)
                    addk(48, 3, 3)
                    add(50, lambda: held.__setitem__("qt", load_tabs(1, "q")))
                    for jt in range(JT):
                        addq(52 + 3 * jt, 1, jt, held)
                elif tq4 < NQ - 1:
                    add(24, lambda tq4=tq4: held.__setitem__(
                        "qt", load_tabs(tq4 + 1, "q")))
                    for jt in range(JT):
                        addq(28 + 3 * jt, tq4 + 1, jt, held)
                if tq4 < NQ - 1:
                    add(44, lambda tq4=tq4: load_mask(tq4 + 1, 0))

                for jt in range(JT):
                    for tkt in range(TT):
                        slot = jt * 16 + tkt
                        if tkt == 0:
                            force(("q", tq4, jt))   # qfq ready for scores
                        if tkt % 4 == 0:
                            force(("k", tkt // 4, jt))  # kf ready for scores
                        elif tkt % 4 == 1 and tkt < 13:
                            # drain the NEXT chunk 3 slots early so its
                            # 8-MM chain + rotary latency is hidden
                            force(("k", tkt // 4 + 1, jt))
                        ps = scp.tile([128, 2, 512], fp32, tag="sc")
                        mb = held[("mk", tq4, tkt // 8)]
                        r = tkt % 8
                        for e in range(2):
                            nc.tensor.matmul(
                                ps[:, e, :],
                                kf[ds(e * 64, 64), jt, ts(tkt, 128)],
                                qfq[ds(e * 64, 64), tq4 % 2, jt, :],
                                start=True, stop=True)
                        pump(2)
                        if tq4 == NQ - 1 and slot >= 36:
                            target = 1 if slot >= 52 else 4
                        elif slot >= 56:
                            target = 8   # taper into the quarter boundary
                        else:
                            target = SKEW
                        while len(pvq) >= target:
                            pop_pv()
                        pump(2)
                        pr = prp.tile([128, 2, 512], bf16, tag="pr")
                        nc.scalar.activation(pr[:, :, :], ps[:, :, :],
                                             EXP, bias=bias_m40[:, :],
                                             scale=0.03125)
                        if (tq4, slot) in MASK_ON_GPSIMD:
                            nc.gpsimd.tensor_tensor(
                                pr[:, :, :], pr[:, :, :],
                                mb[:, r, None, :].broadcast_to((128, 2, 512)),
                                MUL)
                        pvq.append((pr, jt, tkt, (tq4, jt), mb, r, slot))
                        for _ in range(2 if tq4 == NQ - 1 else 1):
                            if sideq:
                                sideq.pop(0)()
                        for fn in inject.get(slot, ()):
                            fn()
                        pump(2)
                        if PACER_EVERY and tq4 > 0 and slot % PACER_EVERY == 0:
                            pacer(1)
                while workq:      # quarter boundary: flush queued pieces
                    pump(100)

            # ---- drain ----------------------------------------------------
            while pvq:
                pop_pv()
            epilogue(st["gkey"], st["accs"])
            while sideq:
                sideq.pop(0)()
            for jo in range(8):
                wo_piece(NQ - 1, jo)
    nc.finalize()
    return nc


def _host_inputs(x, attn_mask, Wq, Wk, Wv, Wo):
    x = np.asarray(x, dtype=np.float32)
    attn_mask = np.asarray(attn_mask)
    Wq = np.asarray(Wq, dtype=np.float32)
    Wk = np.asarray(Wk, dtype=np.float32)
    Wv = np.asarray(Wv, dtype=np.float32)
    Wo = np.asarray(Wo, dtype=np.float32)

    cos, sin, scale = _rot_tables_np()
    cosT, sinT, scaleT = cos.T, sin.T, scale.T            # [D, T]
    # sign-fold for the partition-swap rotate-half: even d rows get -sin
    sgn = np.where(np.arange(D) % 2 == 0, -1.0, 1.0).astype(np.float32)[:, None]
    qcos = np.ascontiguousarray(np.tile(cosT * scaleT, (2, 1)), dtype=np.float32)
    qsin = np.ascontiguousarray(np.tile(sinT * scaleT * sgn, (2, 1)),
                                dtype=np.float32)
    kcos = np.ascontiguousarray(np.tile(cosT / scaleT, (2, 1)), dtype=np.float32)
    ksin = np.ascontiguousarray(np.tile(sinT / scaleT * sgn, (2, 1)),
                                dtype=np.float32)
    ident = np.eye(128, dtype=ml_dtypes.bfloat16)

    in_maps = []
    for b in range(B):
        xTb = np.ascontiguousarray(x[b].T.astype(ml_dtypes.bfloat16))
        mB16 = np.ascontiguousarray(
            (attn_mask[b, 0].T != 0).astype(ml_dtypes.bfloat16))  # {0, 1}
        for g in range(G):
            sl = slice(CG * g, CG * (g + 1))
            Wq_g, Wk_g, Wv_g = Wq[sl], Wk[sl], Wv[sl]
            in_maps.append({
                "xT": xTb,
                "maskB": mB16,
                "wq": np.ascontiguousarray(Wq_g.T.astype(ml_dtypes.bfloat16)),
                "wk": np.ascontiguousarray(Wk_g.T.astype(ml_dtypes.bfloat16)),
                "wv": np.ascontiguousarray(Wv_g.T.astype(ml_dtypes.bfloat16)),
                "wo": np.ascontiguousarray(Wo[:, sl].T.astype(ml_dtypes.bfloat16)),
                "qcos": qcos, "qsin": qsin, "kcos": kcos, "ksin": ksin,
                "ident": ident,
            })
    return in_maps


def kernel(x, attn_mask, Wq, Wk, Wv, Wo):
    from concourse.bass_utils import run_bass_kernel_spmd

    if "nc" not in _CACHE:
        _CACHE["nc"] = _build_bass()
    nc = _CACHE["nc"]

    in_maps = _host_inputs(x, attn_mask, Wq, Wk, Wv, Wo)
    res = run_bass_kernel_spmd(nc, in_maps, core_ids=list(range(NCORES)))
    _CACHE["last_results"] = res

    y = np.empty((B, T, C), dtype=np.float32)
    for b in range(B):
        acc = np.asarray(res.results[2 * b]["yT"], dtype=np.float32) + \
              np.asarray(res.results[2 * b + 1]["yT"], dtype=np.float32)
        y[b] = acc.T
    return y



# revision 137
# speedup vs baseline: 1.0097x; 1.0075x over previous
"""Trainium2 Bass kernel for nn_AttentionLayer (B=4, T=2048, C=1024, H=16, D=64).

Sharding: 8 cores = 4 batches x 2 head-groups (8 heads each). Each core
computes a partial y[b] = out_g @ Wo_g^T; host sums the two group partials
per batch and transposes back.

Single fused pipeline engineered around the per-engine floors: ScalarE exp
(256 x ~1.04us), PE matmul out-element cost (scores+PV+projections ~278us),
and DVE (mask multiplies + rotary + copies ~263us). Design:
  - bf16 throughout; k/q projected per 512-column chunk with xpos rotary
    fused on DVE (+GPSIMD for one multiply), stored as bf16 kf (full T) and
    per-quarter double-buffered qf.
  - scoresT[tk,tq] per head-pair in a double-buffered PSUM pair; exp on
    ScalarE (constant -40 bias, 1/32 scale - no row-max pass needed).
  - mask applied POST-exp as a bf16 {0,1} multiply on DVE (2x mode, mask
    broadcast over the head pair). Its emission is deferred until the pv
    pop SKEW slots later, so a TT stalled on a mask DMA never wedges the
    in-order DVE queue ahead of rotary work, and mask loads get ~20us of
    deadline slack. (GPSIMD cannot touch PSUM on real HW, so all PSUM
    reads - copies, normalize - stay on DVE/ACT.)
  - transposed PV: out[q,0:66] += pr[:,e,qtile].T @ vaug[tk,h,0:66] (cols
    64:66 ones = softmax denominator); 16 accumulators (qt-major: s=qt*2+e)
    packed in 3 PSUM banks with per-bank start/stop flags. Epilogue: one
    reciprocal + ONE broadcast tensor_tensor normalizes 7 accumulators at
    a stroke; PE transposes read adjacent (e0,e1) pairs; Wo per 128-row
    output block, output staged through a 4-deep ysb ring.
  - DMA discipline: every dma_start pays ~625ns on the single HWDGE
    descriptor-gen device and transfers drain one DMA_ENGINES queue in
    issue order, so the prologue issues exactly in dependency order
    (x0/wk interleaved 2-chunk pieces, ktabs0, wq, qtabs0, x1, ktabs1,
    x2, ktabs2, wv, x3, ktabs3, masks, ident, wo) and bulk loads never
    preempt the first-scores path.
  - quarter-0 carries all k/v projections (every later quarter re-sweeps
    all tk): injected into attention slots sequenced by DMA arrival, with
    k-chunk generators force-drained 3 slots before first use to hide the
    8-MM chain + rotary latency. The pv backlog tapers near quarter ends
    so deferred work doesn't dump into the next quarter's first slots.
  - warm-up pacer matmuls ride out the PE p-state ramp (post-idle matmuls
    run at half clock for ~3us) during the initial weight DMA.
"""

import numpy as np
import ml_dtypes

B, T, C, H, D = 4, 2048, 1024, 16, 64
G = 2                 # head groups (tensor parallel)
NCORES = B * G
CG = C // G           # 512 channels per group
JT = CG // 128        # 4 head-pairs per core
CCH = C // 128        # 8 contraction chunks
TT = T // 128         # 16 tk-tiles
NQ = 4                # tq quarters of 512
THETA = 10000.0
SCALE_BASE = 512.0

# scheduling knobs
SKEW = 19             # pv matmuls trail the scores/exp/mask stream (slots)
PR_BUFS = 23
WARMUP_PACERS = 10
PACER_EVERY = 0       # if >0: one pacer MM every N slots in quarters 1-3

_CACHE = {}


def _rot_tables_np():
    inv_freq = 1.0 / (THETA ** (np.arange(0, D, 2, dtype=np.float32) / D))
    seq = np.arange(T, dtype=np.float32)
    freqs = seq[:, None] * inv_freq[None, :]
    freqs = np.repeat(freqs, 2, axis=-1)                    # [T, D]
    base = (np.arange(0, D, 2, dtype=np.float32) + 0.4 * D) / (1.4 * D)
    power = (seq - T // 2) / SCALE_BASE
    scale = base[None, :] ** power[:, None]
    scale = np.repeat(scale, 2, axis=-1)                    # [T, D]
    return np.cos(freqs), np.sin(freqs), scale.astype(np.float32)


def _build_bass():
    import concourse.bass as bass
    import concourse.bacc as bacc
    import concourse.mybir as mybir
    import concourse.tile as tile
    from concourse.bass import ts, ds

    fp32 = mybir.dt.float32
    bf16 = mybir.dt.bfloat16
    MUL = mybir.AluOpType.mult
    ADD = mybir.AluOpType.add
    EXP = mybir.ActivationFunctionType.Exp

    nc = bacc.Bacc(None)

    xT = nc.dram_tensor("xT", [C, T], bf16, kind="ExternalInput")
    wq = nc.dram_tensor("wq", [C, CG], bf16, kind="ExternalInput")
    wk = nc.dram_tensor("wk", [C, CG], bf16, kind="ExternalInput")
    wv = nc.dram_tensor("wv", [C, CG], bf16, kind="ExternalInput")
    wo = nc.dram_tensor("wo", [CG, C], bf16, kind="ExternalInput")
    qcos = nc.dram_tensor("qcos", [128, T], fp32, kind="ExternalInput")
    qsin = nc.dram_tensor("qsin", [128, T], fp32, kind="ExternalInput")
    kcos = nc.dram_tensor("kcos", [128, T], fp32, kind="ExternalInput")
    ksin = nc.dram_tensor("ksin", [128, T], fp32, kind="ExternalInput")
    maskB = nc.dram_tensor("maskB", [T, T], bf16, kind="ExternalInput")
    ident_d = nc.dram_tensor("ident", [128, 128], bf16, kind="ExternalInput")
    yT = nc.dram_tensor("yT", [C, T], fp32, kind="ExternalOutput")

    xT_r = xT.rearrange("(cc p) t -> p cc t", p=128)        # [128, 8, T]
    maskB_r = maskB.rearrange("(tk p) q -> p tk q", p=128)  # [128, 16, T]
    SWAPM = [i + 1 - 2 * (i % 2) for i in range(32)]

    # pv accumulator group = one head-pair (jt): 8 accumulators [128, 66]
    # in 2 PSUM banks (pvA rows 0..6, pvB row 0 = s7). The accumulate
    # zero-region is a whole bank, so only the chronologically-first MM per
    # bank starts the group and the last stops it.
    PV_START = {0, 7}
    PV_STOP = {5, 7}
    MASK_ON_GPSIMD = set()  # (quarter, slot) pairs: mask-mult on Pool

    with tile.TileContext(nc) as tc:
        with (
            tc.tile_pool(name="persist", bufs=1) as persist,
            tc.tile_pool(name="xp", bufs=4) as xp,
            tc.tile_pool(name="ktabp", bufs=4) as ktabp,
            tc.tile_pool(name="qtabp", bufs=1) as qtabp,
            tc.tile_pool(name="maskp", bufs=3) as maskp,
            tc.tile_pool(name="prp", bufs=PR_BUFS) as prp,
            tc.tile_pool(name="rotp", bufs=3) as rotp,
            tc.tile_pool(name="stgp", bufs=2) as stgp,
            tc.tile_pool(name="recp", bufs=6) as recp,
            tc.tile_pool(name="ysbp", bufs=4) as ysbp,
            tc.tile_pool(name="scp", bufs=2, space="PSUM") as scp,
            tc.tile_pool(name="pvp", bufs=2, space="PSUM") as pvp,
            tc.tile_pool(name="tailp", bufs=2, space="PSUM") as tailp,
        ):
            kf = persist.tile([128, JT, T], bf16, tag="kf")
            qfq = persist.tile([128, 2, JT, 512], bf16, tag="qfq")
            vaug = persist.tile([128, TT, 8, 66], bf16, tag="vaug")
            wq_sb = persist.tile([128, CCH, CG], bf16, tag="wq")
            # wk shares the mask ring: k projections finish in quarter 0,
            # after which its 8KB slot recycles into mask buffers.
            wk_sb = maskp.tile([128, CCH, CG], bf16, tag="mask", name="wk_sb")
            wv_sb = persist.tile([128, CCH, CG], bf16, tag="wv")
            wo_sb = persist.tile([128, JT, C], bf16, tag="wo")
            oth = persist.tile([128, JT, 512], bf16, tag="oth")
            ident = persist.tile([128, 128], bf16, tag="ident")
            bias_m40 = persist.tile([128, 1], fp32, tag="biasm40")
            dmy_a = persist.tile([128, 128], bf16, tag="dmya")
            dmy_b = persist.tile([128, 256], bf16, tag="dmyb")
            dmy_s = persist.tile([128, 1], fp32, tag="dmys")

            # ---- t=0 warm-up: pacers ride out the p-state ramp + DMA head;
            # a dummy activation pulls the exp table load forward.
            nc.vector.memset(dmy_a[:], 0.0)
            nc.vector.memset(dmy_b[:], 0.0)
            nc.vector.memset(bias_m40[:], -40.0)
            nc.vector.memset(vaug[:, :, :, 64:66], 1.0)
            nc.scalar.activation(dmy_s[:], bias_m40[:], EXP,
                                 bias=bias_m40[:, :], scale=0.0)
            pacer_ps = tailp.tile([128, 512], fp32, tag="tail")
            for _ in range(WARMUP_PACERS):
                nc.tensor.matmul(pacer_ps[:, 0:256], dmy_a[:], dmy_b[:],
                                 start=True, stop=True)

            def pacer(n):
                pps = tailp.tile([128, 512], fp32, tag="tail")
                for _ in range(n):
                    nc.tensor.matmul(pps[:, 0:256], dmy_a[:], dmy_b[:],
                                     start=True, stop=True)

            # ---- weight DMA (k path first - it gates the first scores) ---
            # Two half-granularity DMAs per weight: each dma_start pays
            # ~625ns on the single HWDGE descriptor-gen device, so fewer,
            # bigger transfers win; halves keep the prelude MM chain
            # streaming. Bulk loads go through gpsimd (SWDGE) which has its
            # own desc-gen path and leaves HWDGE free for the urgent loads.
            def wdma(w_sb, w_dr, n=2):
                r = w_dr.rearrange("(cc p) j -> p cc j", p=128)
                step = CCH // n
                for i in range(0, CCH, step):
                    nc.sync.dma_start(out=w_sb[:, i:i + step, :],
                                      in_=r[:, i:i + step, :])

            def load_x(tcx):
                tsl = ds(tcx * 512, 512)
                xtc = xp.tile([128, CCH, 512], bf16, tag="x")
                nc.sync.dma_start(out=xtc[:], in_=xT_r[:, :, tsl])
                return xtc

            def load_tabs(tcx, which, eng=None):
                tsl = ds(tcx * 512, 512)
                srcs = {"q": (("tqc", qcos), ("tqs", qsin)),
                        "k": (("tkc", kcos), ("tks", ksin))}[which]
                pool = qtabp if which == "q" else ktabp
                eng = eng or nc.sync
                tabs = []
                for nm, dr in srcs:
                    t = pool.tile([128, 512], fp32, tag=nm)
                    eng.dma_start(out=t[:], in_=dr[:, tsl])
                    tabs.append(t)
                return tabs

            def rotary(ps, tabs, dst, eng2=None):
                # dst = ps*cos + pairswap(ps)*sin   (sign folded into sin)
                swp = rotp.tile([128, 512], fp32, tag="rt")
                nc.vector.stream_shuffle(swp[:], ps[:], SWAPM)
                t1 = rotp.tile([128, 512], fp32, tag="rt")
                nc.vector.tensor_tensor(t1[:], ps[:], tabs[0][:], MUL)
                t2 = rotp.tile([128, 512], fp32, tag="rt")
                (eng2 or nc.gpsimd).tensor_tensor(t2[:], swp[:], tabs[1][:], MUL)
                nc.vector.tensor_tensor(dst, t1[:], t2[:], ADD)

            # ---- prelude: x resident (all 4 chunks), k0/q0 for jt0 only;
            # everything else is injected into attention slots.
            held = {}

            def load_mask(q, half, eng=None):
                m = maskp.tile([128, 8, 512], bf16, tag="mask")
                (eng or nc.sync).dma_start(
                    out=m[:], in_=maskB_r[:, half * 8:half * 8 + 8,
                                          ds(q * 512, 512)])
                held[("mk", q, half)] = m

            # All prologue DMAs ride the sync HWDGE queue; transfers drain
            # the single DMA_ENGINES device in exactly this order. Sequence
            # by downstream need: first-scores path (x0,wk,ktabs0,qtabs0),
            # then mask half 0 (unblocks the slot-0 mask multiply on DVE),
            # x1 + wq, wv (v-tile pieces get pumped from slot ~1), k-tabs
            # and x chunks interleaved by deadline, mask half 1 before jt0's
            # tkt-8 multiply, wo/ident last.
            # interleave x0/wk piece DMAs so the k0 chain streams cc-by-cc
            xtc0 = xp.tile([128, CCH, 512], bf16, tag="x", name="xtc0")
            wk_r = wk.rearrange("(cc p) j -> p cc j", p=128)
            for i in range(0, CCH, 2):
                nc.sync.dma_start(out=xtc0[:, i:i + 2, :],
                                  in_=xT_r[:, i:i + 2, ds(0, 512)])
                nc.sync.dma_start(out=wk_sb[:, i:i + 2, :],
                                  in_=wk_r[:, i:i + 2, :])
            xall = [xtc0]
            ktabs = {0: load_tabs(0, "k")}
            wdma(wq_sb, wq, n=4)
            qtabs0 = load_tabs(0, "q")
            xall += [load_x(1)]
            ktabs[1] = load_tabs(1, "k")
            xall += [load_x(2)]
            ktabs[2] = load_tabs(2, "k")
            nc.sync.dma_start(
                out=wv_sb[:], in_=wv.rearrange("(cc p) j -> p cc j", p=128))
            xall += [load_x(3)]
            ktabs[3] = load_tabs(3, "k")
            load_mask(0, 0)
            load_mask(0, 1)
            nc.sync.dma_start(out=ident[:], in_=ident_d[:])
            nc.sync.dma_start(
                out=wo_sb[:], in_=wo.rearrange("(cc p) j -> p cc j", p=128))

            def proj_gen(w_sb, tcx, jt, fini):
                # generator piece: ~2 matmuls per step so injected work
                # never blocks the in-order PE stream for long
                ps = tailp.tile([128, 512], fp32, tag="tail")
                for h in range(4):
                    for cc in (2 * h, 2 * h + 1):
                        nc.tensor.matmul(ps[:], w_sb[:, cc, ts(jt, 128)],
                                         xall[tcx][:, cc, :],
                                         start=(cc == 0),
                                         stop=(cc == CCH - 1))
                    yield
                fini(ps)

            def kproj_gen(tcx, jt, eng2=None):
                return proj_gen(
                    wk_sb, tcx, jt,
                    lambda ps: rotary(ps[:], ktabs[tcx],
                                      kf[:, jt, ds(tcx * 512, 512)], eng2))

            def qproj_gen(tcx, tabs, jt, eng2=None):
                def fini(ps):
                    t = tabs["qt"] if isinstance(tabs, dict) else tabs
                    rotary(ps[:], t, qfq[:, tcx % 2, jt, :], eng2)
                return proj_gen(wq_sb, tcx, jt, fini)

            def kproj(tcx, jt, eng2=None):
                for _ in kproj_gen(tcx, jt, eng2):
                    pass

            def qproj(tcx, tabs, jt, eng2=None):
                for _ in qproj_gen(tcx, tabs, jt, eng2):
                    pass

            # jt0's chunk-0 projections in the prologue (k chain first: wk+x0
            # land before wq); jt1-3's ride early quarter-0 slots, which have
            # PE slack until PV kicks in at slot SKEW. Pacers bridge the PE
            # gap between the k chain and wq's arrival so the p-state ramp
            # never resets.
            kproj(0, 0, eng2=nc.vector)
            # the wq transfer is still in flight here; fill the window with
            # jt1-3's chunk-0 k chains instead of dummy pacers (their
            # rotaries also hide the first-scores wait on wq/qtabs0).
            kproj(0, 1, eng2=nc.vector)
            kproj(0, 2, eng2=nc.vector)
            kproj(0, 3, eng2=nc.vector)
            qproj(0, qtabs0, 0, eng2=nc.vector)

            # ---- injected side-work pieces -------------------------------
            def vtile_gen(tti_global):
                xt = xall[tti_global // 4]
                tsl = ts(tti_global % 4, 128)
                ps = tailp.tile([128, 8, 64], fp32, tag="tail")
                for h in range(4):
                    for cc in (2 * h, 2 * h + 1):
                        nc.tensor.matmul(ps[:, :, :], xt[:, cc, tsl],
                                         wv_sb[:, cc, :],
                                         start=(cc == 0),
                                         stop=(cc == CCH - 1))
                    yield
                nc.scalar.copy(vaug[:, tti_global, :, 0:64], ps[:, :, :])

            # ---- attention machinery -------------------------------------
            st = {"accs": None, "gkey": None, "y_written": 0}
            sideq = []                 # deferred small thunks (epilogue, wo)

            def acc_slice(accs, s):
                if s < 7:
                    return accs[0][:, s, :]
                return accs[1][:, s - 7, :]

            def epilogue(gkey, accs):
                tq4, jt = gkey
                rA = recp.tile([128, 8], fp32, tag="rec")
                nc.vector.reciprocal(rA[:, 0:7, None], accs[0][:, 0:7, 64:65])
                rB = recp.tile([128, 8], fp32, tag="rec")
                nc.vector.reciprocal(rB[:, 0:1, None], accs[1][:, 0:1, 64:65])

                # normalize all 7 pvA accumulators in ONE tensor_tensor with
                # the reciprocals broadcast along d (vs 7 serial TSPs); the
                # transposes then gather (s=qt, s=qt+4) as a strided slice.
                stg = stgp.tile([128, 8, 64], bf16, tag="stg")
                nc.vector.tensor_tensor(
                    stg[:, 0:7, :], accs[0][:, 0:7, 0:64],
                    rA[:, 0:7, None].broadcast_to((128, 7, 64)), MUL)
                nc.vector.tensor_scalar_mul(
                    stg[:, 7, :], accs[1][:, 0, 0:64], rB[:, 0:1])

                tb = tailp.tile([128, 4, 128], bf16, tag="tail")
                for qt in range(4):
                    nc.tensor.transpose(
                        tb[:, qt, :],
                        stg[:, 2 * qt:2 * qt + 2, :], ident[:])
                    nc.vector.tensor_copy(
                        oth[:, jt, ds(qt * 128, 128)], tb[:, qt, :])

            def wo_piece(tq4, jo):
                py = tailp.tile([128, 512], fp32, tag="tail")
                for cc in range(JT):
                    nc.tensor.matmul(py[:], wo_sb[:, cc, ts(jo, 128)],
                                     oth[:, cc, :],
                                     start=(cc == 0), stop=(cc == JT - 1))
                ysb = ysbp.tile([128, 512], fp32, tag="ysb")
                nc.vector.tensor_copy(ysb[:], py[:])
                nc.sync.dma_start(out=yT[ts(jo, 128), ds(tq4 * 512, 512)],
                                  in_=ysb[:])

            def ensure_group(gkey):
                if st["gkey"] == gkey:
                    return
                if st["gkey"] is not None:
                    epilogue(st["gkey"], st["accs"])
                    fin_tq4, fin_jt = st["gkey"]
                    if fin_jt == JT - 1:   # quarter's oth complete -> Wo
                        for jo in range(8):
                            sideq.append(
                                lambda tq4=fin_tq4, jo=jo: wo_piece(tq4, jo))
                pvA = pvp.tile([128, 7, 66], fp32, tag="pv")
                pvB = pvp.tile([128, 1, 66], fp32, tag="pv")
                st["accs"] = (pvA, pvB)
                st["gkey"] = gkey

            def emit_pv(pr, jt, tkt, gkey):
                ensure_group(gkey)
                for e in range(2):
                    h = jt * 2 + e
                    for qt in range(4):
                        s = qt * 2 + e
                        nc.tensor.matmul(
                            acc_slice(st["accs"], s),
                            pr[:, e, ts(qt, 128)],
                            vaug[:, tkt, h, 0:66],
                            start=(tkt == 0 and s in PV_START),
                            stop=(tkt == TT - 1 and s in PV_STOP),
                            skip_group_check=True)

            pvq = []                  # [(pr, jt, tkt, gkey, mb, r, slot), ...]
            workq = []                # [(key, generator)] fine-grained pieces

            def pop_pv():
                # DVE mask multiplies are emitted HERE, SKEW slots after the
                # exp: rotaries emitted near the exp slot never queue on DVE
                # behind a TT stalled on a mask DMA, and mask loads gain
                # ~20us of deadline slack. Pool-offloaded multiplies were
                # already emitted at exp time (their latency hides in the
                # skew and they never block DVE).
                pr, jt, tkt, gkey, mb, r, slot = pvq.pop(0)
                force(("v", tkt))           # vaug ready for pv
                if (gkey[0], slot) not in MASK_ON_GPSIMD:
                    nc.vector.tensor_tensor(
                        pr[:, :, :], pr[:, :, :],
                        mb[:, r, None, :].broadcast_to((128, 2, 512)), MUL)
                emit_pv(pr, jt, tkt, gkey)

            def pump(nsteps):
                while nsteps > 0 and workq:
                    key, g = workq[0]
                    try:
                        next(g)
                        nsteps -= 1
                    except StopIteration:
                        workq.pop(0)

            def force(key):
                # emission-order deadline: finish FIFO head pieces up to and
                # including `key` so dependent reads are emitted after writes
                while any(k == key for k, _ in workq):
                    k0, g = workq[0]
                    try:
                        next(g)
                    except StopIteration:
                        workq.pop(0)

            # ---- main loop ------------------------------------------------

            for tq4 in range(NQ):
                qsl = ds(tq4 * 512, 512)
                if tq4 > 0:
                    load_mask(tq4, 1)   # first half was prefetched

                inject = {}

                def add(slot, fn):
                    inject.setdefault(slot, []).append(fn)

                def addg(slot, key, mk):
                    # queue a generator piece at slot; tracked for deadlines
                    def starter():
                        workq.append((key, mk()))
                    add(slot, starter)

                def addk(slot, tcx, jt):
                    addg(slot, ("k", tcx, jt), lambda: kproj_gen(tcx, jt))

                def addq(slot, tcx, jt, tabs):
                    addg(slot, ("q", tcx, jt),
                         lambda: qproj_gen(tcx, tabs, jt))

                def addv(slot, i):
                    addg(slot, ("v", i), lambda: vtile_gen(i))

                if tq4 == 0:
                    # remaining k chunks per jt (deadline slot jt*16+4c),
                    # k0/q0 projections for jt 1-3 (deadline jt*16), all 16
                    # v tiles (deadline tt+SKEW), q quarter 1 late.
                    # workq is FIFO and pump pops the head only, so adds are
                    # sequenced by DMA arrival: x0/wq-based first, then the
                    # x1/x2/x3-gated chunks as close to their force slot as
                    # possible so a not-yet-landed DMA never wedges the head.
                    addk(0, 1, 0)                    # x1-based
                    addq(3, 0, 1, qtabs0)
                    addk(4, 2, 0)                    # x2-based
                    addq(7, 0, 2, qtabs0)
                    addv(7, 0)                       # wv-gated
                    addk(9, 3, 0)                    # x3-based
                    addv(9, 1)
                    addq(11, 0, 3, qtabs0)
                    addv(12, 2)
                    addv(13, 3)
                    addk(14, 1, 1)
                    addv(15, 4)
                    addk(16, 2, 1)
                    addv(17, 5)
                    addk(18, 3, 1)
                    addv(19, 6)
                    addv(20, 7)
                    addv(22, 8)                      # v8-11 (x2-based)
                    addv(23, 9)
                    addv(24, 10)
                    addv(25, 11)
                    addv(26, 12)                     # v12-15 (x3-based)
                    addv(27, 13)
                    addv(28, 14)
                    addv(29, 15)
                    addk(31, 1, 2)
                    addk(34, 2, 2)
                    addk(37, 3, 2)
                    addk(44, 1, 3)
                    addk(46, 2, 3)
                    addk(48, 3, 3)
                    add(50, lambda: held.__setitem__("qt", load_tabs(1, "q")))
                    for jt in range(JT):
                        addq(52 + 3 * jt, 1, jt, held)
                elif tq4 < NQ - 1:
                    add(24, lambda tq4=tq4: held.__setitem__(
                        "qt", load_tabs(tq4 + 1, "q")))
                    for jt in range(JT):
                        addq(28 + 3 * jt, tq4 + 1, jt, held)
                if tq4 < NQ - 1:
                    add(44, lambda tq4=tq4: load_mask(tq4 + 1, 0))

                for jt in range(JT):
                    for tkt in range(TT):
                        slot = jt * 16 + tkt
                        if tkt == 0:
                            force(("q", tq4, jt))   # qfq ready for scores
                        if tkt % 4 == 0:
                            force(("k", tkt // 4, jt))  # kf ready for scores
                        elif tkt % 4 == 1 and tkt < 13:
                            # drain the NEXT chunk 3 slots early so its
                            # 8-MM chain + rotary latency is hidden
                            force(("k", tkt // 4 + 1, jt))
                        ps = scp.tile([128, 2, 512], fp32, tag="sc")
                        mb = held[("mk", tq4, tkt // 8)]
                        r = tkt % 8
                        for e in range(2):
                            nc.tensor.matmul(
                                ps[:, e, :],
                                kf[ds(e * 64, 64), jt, ts(tkt, 128)],
                                qfq[ds(e * 64, 64), tq4 % 2, jt, :],
                                start=True, stop=True)
                        pump(2)
                        if tq4 == NQ - 1 and slot >= 40:
                            target = 2 if slot >= 56 else 8
                        elif slot >= 56:
                            target = 14   # taper into the quarter boundary
                        else:
                            target = SKEW
                        while len(pvq) >= target:
                            pop_pv()
                        pump(2)
                        pr = prp.tile([128, 2, 512], bf16, tag="pr")
                        nc.scalar.activation(pr[:, :, :], ps[:, :, :],
                                             EXP, bias=bias_m40[:, :],
                                             scale=0.03125)
                        if (tq4, slot) in MASK_ON_GPSIMD:
                            nc.gpsimd.tensor_tensor(
                                pr[:, :, :], pr[:, :, :],
                                mb[:, r, None, :].broadcast_to((128, 2, 512)),
                                MUL)
                        pvq.append((pr, jt, tkt, (tq4, jt), mb, r, slot))
                        for _ in range(2 if tq4 == NQ - 1 else 1):
                            if sideq:
                                sideq.pop(0)()
                        for fn in inject.get(slot, ()):
                            fn()
                        pump(2)
                        if PACER_EVERY and tq4 > 0 and slot % PACER_EVERY == 0:
                            pacer(1)
                while workq:      # quarter boundary: flush queued pieces
                    pump(100)

            # ---- drain ----------------------------------------------------
            while pvq:
                pop_pv()
            epilogue(st["gkey"], st["accs"])
            while sideq:
                sideq.pop(0)()
            for jo in range(8):
                wo_piece(NQ - 1, jo)
    nc.finalize()
    return nc


def _host_inputs(x, attn_mask, Wq, Wk, Wv, Wo):
    x = np.asarray(x, dtype=np.float32)
    attn_mask = np.asarray(attn_mask)
    Wq = np.asarray(Wq, dtype=np.float32)
    Wk = np.asarray(Wk, dtype=np.float32)
    Wv = np.asarray(Wv, dtype=np.float32)
    Wo = np.asarray(Wo, dtype=np.float32)

    cos, sin, scale = _rot_tables_np()
    cosT, sinT, scaleT = cos.T, sin.T, scale.T            # [D, T]
    # sign-fold for the partition-swap rotate-half: even d rows get -sin
    sgn = np.where(np.arange(D) % 2 == 0, -1.0, 1.0).astype(np.float32)[:, None]
    qcos = np.ascontiguousarray(np.tile(cosT * scaleT, (2, 1)), dtype=np.float32)
    qsin = np.ascontiguousarray(np.tile(sinT * scaleT * sgn, (2, 1)),
                                dtype=np.float32)
    kcos = np.ascontiguousarray(np.tile(cosT / scaleT, (2, 1)), dtype=np.float32)
    ksin = np.ascontiguousarray(np.tile(sinT / scaleT * sgn, (2, 1)),
                                dtype=np.float32)
    ident = np.eye(128, dtype=ml_dtypes.bfloat16)

    in_maps = []
    for b in range(B):
        xTb = np.ascontiguousarray(x[b].T.astype(ml_dtypes.bfloat16))
        mB16 = np.ascontiguousarray(
            (attn_mask[b, 0].T != 0).astype(ml_dtypes.bfloat16))  # {0, 1}
        for g in range(G):
            sl = slice(CG * g, CG * (g + 1))
            Wq_g, Wk_g, Wv_g = Wq[sl], Wk[sl], Wv[sl]
            in_maps.append({
                "xT": xTb,
                "maskB": mB16,
                "wq": np.ascontiguousarray(Wq_g.T.astype(ml_dtypes.bfloat16)),
                "wk": np.ascontiguousarray(Wk_g.T.astype(ml_dtypes.bfloat16)),
                "wv": np.ascontiguousarray(Wv_g.T.astype(ml_dtypes.bfloat16)),
                "wo": np.ascontiguousarray(Wo[:, sl].T.astype(ml_dtypes.bfloat16)),
                "qcos": qcos, "qsin": qsin, "kcos": kcos, "ksin": ksin,
                "ident": ident,
            })
    return in_maps


def kernel(x, attn_mask, Wq, Wk, Wv, Wo):
    from concourse.bass_utils import run_bass_kernel_spmd

    if "nc" not in _CACHE:
        _CACHE["nc"] = _build_bass()
    nc = _CACHE["nc"]

    in_maps = _host_inputs(x, attn_mask, Wq, Wk, Wv, Wo)
    res = run_bass_kernel_spmd(nc, in_maps, core_ids=list(range(NCORES)))
    _CACHE["last_results"] = res

    y = np.empty((B, T, C), dtype=np.float32)
    for b in range(B):
        acc = np.asarray(res.results[2 * b]["yT"], dtype=np.float32) + \
              np.asarray(res.results[2 * b + 1]["yT"], dtype=np.float32)
        y[b] = acc.T
    return y



# revision 142
# speedup vs baseline: 1.0125x; 1.0027x over previous
"""Trainium2 Bass kernel for nn_AttentionLayer (B=4, T=2048, C=1024, H=16, D=64).

Sharding: 8 cores = 4 batches x 2 head-groups (8 heads each). Each core
computes a partial y[b] = out_g @ Wo_g^T; host sums the two group partials
per batch and transposes back.

Single fused pipeline engineered around the per-engine floors: ScalarE exp
(256 x ~1.04us), PE matmul out-element cost (scores+PV+projections ~278us),
and DVE (mask multiplies + rotary + copies ~263us). Design:
  - bf16 throughout; k/q projected per 512-column chunk with xpos rotary
    fused on DVE (+GPSIMD for one multiply), stored as bf16 kf (full T) and
    per-quarter double-buffered qf.
  - scoresT[tk,tq] per head-pair in a double-buffered PSUM pair; exp on
    ScalarE (constant -40 bias, 1/32 scale - no row-max pass needed).
  - mask applied POST-exp as a bf16 {0,1} multiply on DVE (2x mode, mask
    broadcast over the head pair). Its emission is deferred until the pv
    pop SKEW slots later, so a TT stalled on a mask DMA never wedges the
    in-order DVE queue ahead of rotary work, and mask loads get ~20us of
    deadline slack. (GPSIMD cannot touch PSUM on real HW, so all PSUM
    reads - copies, normalize - stay on DVE/ACT.)
  - transposed PV: out[q,0:66] += pr[:,e,qtile].T @ vaug[tk,h,0:66] (cols
    64:66 ones = softmax denominator); 16 accumulators (qt-major: s=qt*2+e)
    packed in 3 PSUM banks with per-bank start/stop flags. Epilogue: one
    reciprocal + ONE broadcast tensor_tensor normalizes 7 accumulators at
    a stroke; PE transposes read adjacent (e0,e1) pairs; Wo per 128-row
    output block, output staged through a 4-deep ysb ring.
  - DMA discipline: every dma_start pays ~625ns on the single HWDGE
    descriptor-gen device and transfers drain one DMA_ENGINES queue in
    issue order, so the prologue issues exactly in dependency order
    (x0/wk interleaved 2-chunk pieces, ktabs0, wq, qtabs0, x1, ktabs1,
    x2, ktabs2, wv, x3, ktabs3, masks, ident, wo) and bulk loads never
    preempt the first-scores path.
  - quarter-0 carries all k/v projections (every later quarter re-sweeps
    all tk): injected into attention slots sequenced by DMA arrival, with
    k-chunk generators force-drained 3 slots before first use to hide the
    8-MM chain + rotary latency. The pv backlog tapers near quarter ends
    so deferred work doesn't dump into the next quarter's first slots.
  - warm-up pacer matmuls ride out the PE p-state ramp (post-idle matmuls
    run at half clock for ~3us) during the initial weight DMA.
"""

import numpy as np
import ml_dtypes

B, T, C, H, D = 4, 2048, 1024, 16, 64
G = 2                 # head groups (tensor parallel)
NCORES = B * G
CG = C // G           # 512 channels per group
JT = CG // 128        # 4 head-pairs per core
CCH = C // 128        # 8 contraction chunks
TT = T // 128         # 16 tk-tiles
NQ = 4                # tq quarters of 512
THETA = 10000.0
SCALE_BASE = 512.0

# scheduling knobs
SKEW = 19             # pv matmuls trail the scores/exp/mask stream (slots)
PR_BUFS = 23
WARMUP_PACERS = 10
PACER_EVERY = 0       # if >0: one pacer MM every N slots in quarters 1-3

_CACHE = {}


def _rot_tables_np():
    inv_freq = 1.0 / (THETA ** (np.arange(0, D, 2, dtype=np.float32) / D))
    seq = np.arange(T, dtype=np.float32)
    freqs = seq[:, None] * inv_freq[None, :]
    freqs = np.repeat(freqs, 2, axis=-1)                    # [T, D]
    base = (np.arange(0, D, 2, dtype=np.float32) + 0.4 * D) / (1.4 * D)
    power = (seq - T // 2) / SCALE_BASE
    scale = base[None, :] ** power[:, None]
    scale = np.repeat(scale, 2, axis=-1)                    # [T, D]
    return np.cos(freqs), np.sin(freqs), scale.astype(np.float32)


def _build_bass():
    import concourse.bass as bass
    import concourse.bacc as bacc
    import concourse.mybir as mybir
    import concourse.tile as tile
    from concourse.bass import ts, ds

    fp32 = mybir.dt.float32
    bf16 = mybir.dt.bfloat16
    MUL = mybir.AluOpType.mult
    ADD = mybir.AluOpType.add
    EXP = mybir.ActivationFunctionType.Exp

    nc = bacc.Bacc(None)

    xT = nc.dram_tensor("xT", [C, T], bf16, kind="ExternalInput")
    wq = nc.dram_tensor("wq", [C, CG], bf16, kind="ExternalInput")
    wk = nc.dram_tensor("wk", [C, CG], bf16, kind="ExternalInput")
    wv = nc.dram_tensor("wv", [C, CG], bf16, kind="ExternalInput")
    wo = nc.dram_tensor("wo", [CG, C], bf16, kind="ExternalInput")
    qcos = nc.dram_tensor("qcos", [128, T], bf16, kind="ExternalInput")
    qsin = nc.dram_tensor("qsin", [128, T], bf16, kind="ExternalInput")
    kcos = nc.dram_tensor("kcos", [128, T], bf16, kind="ExternalInput")
    ksin = nc.dram_tensor("ksin", [128, T], bf16, kind="ExternalInput")
    maskB = nc.dram_tensor("maskB", [T, T], bf16, kind="ExternalInput")
    ident_d = nc.dram_tensor("ident", [128, 128], bf16, kind="ExternalInput")
    yT = nc.dram_tensor("yT", [C, T], fp32, kind="ExternalOutput")

    xT_r = xT.rearrange("(cc p) t -> p cc t", p=128)        # [128, 8, T]
    maskB_r = maskB.rearrange("(tk p) q -> p tk q", p=128)  # [128, 16, T]
    SWAPM = [i + 1 - 2 * (i % 2) for i in range(32)]

    # pv accumulator group = one head-pair (jt): 8 accumulators [128, 66]
    # in 2 PSUM banks (pvA rows 0..6, pvB row 0 = s7). The accumulate
    # zero-region is a whole bank, so only the chronologically-first MM per
    # bank starts the group and the last stops it.
    PV_START = {0, 7}
    PV_STOP = {5, 7}
    MASK_ON_GPSIMD = set()  # (quarter, slot) pairs: mask-mult on Pool

    with tile.TileContext(nc) as tc:
        with (
            tc.tile_pool(name="persist", bufs=1) as persist,
            tc.tile_pool(name="xp", bufs=4) as xp,
            tc.tile_pool(name="ktabp", bufs=4) as ktabp,
            tc.tile_pool(name="qtabp", bufs=1) as qtabp,
            tc.tile_pool(name="maskp", bufs=3) as maskp,
            tc.tile_pool(name="prp", bufs=PR_BUFS) as prp,
            tc.tile_pool(name="rotp", bufs=3) as rotp,
            tc.tile_pool(name="stgp", bufs=2) as stgp,
            tc.tile_pool(name="recp", bufs=6) as recp,
            tc.tile_pool(name="ysbp", bufs=4) as ysbp,
            tc.tile_pool(name="scp", bufs=2, space="PSUM") as scp,
            tc.tile_pool(name="pvp", bufs=2, space="PSUM") as pvp,
            tc.tile_pool(name="tailp", bufs=2, space="PSUM") as tailp,
        ):
            kf = persist.tile([128, JT, T], bf16, tag="kf")
            qfq = persist.tile([128, 2, JT, 512], bf16, tag="qfq")
            vaug = persist.tile([128, TT, 8, 66], bf16, tag="vaug")
            wq_sb = persist.tile([128, CCH, CG], bf16, tag="wq")
            # wk shares the mask ring: k projections finish in quarter 0,
            # after which its 8KB slot recycles into mask buffers.
            wk_sb = maskp.tile([128, CCH, CG], bf16, tag="mask", name="wk_sb")
            wv_sb = persist.tile([128, CCH, CG], bf16, tag="wv")
            wo_sb = persist.tile([128, JT, C], bf16, tag="wo")
            oth = persist.tile([128, JT, 512], bf16, tag="oth")
            ident = persist.tile([128, 128], bf16, tag="ident")
            bias_m40 = persist.tile([128, 1], fp32, tag="biasm40")
            dmy_a = persist.tile([128, 128], bf16, tag="dmya")
            dmy_b = persist.tile([128, 256], bf16, tag="dmyb")
            dmy_s = persist.tile([128, 1], fp32, tag="dmys")

            # ---- t=0 warm-up: pacers ride out the p-state ramp + DMA head;
            # a dummy activation pulls the exp table load forward.
            nc.vector.memset(dmy_a[:], 0.0)
            nc.vector.memset(dmy_b[:], 0.0)
            nc.vector.memset(bias_m40[:], -40.0)
            nc.vector.memset(vaug[:, :, :, 64:66], 1.0)
            nc.scalar.activation(dmy_s[:], bias_m40[:], EXP,
                                 bias=bias_m40[:, :], scale=0.0)
            pacer_ps = tailp.tile([128, 512], fp32, tag="tail")
            for _ in range(WARMUP_PACERS):
                nc.tensor.matmul(pacer_ps[:, 0:256], dmy_a[:], dmy_b[:],
                                 start=True, stop=True)

            def pacer(n):
                pps = tailp.tile([128, 512], fp32, tag="tail")
                for _ in range(n):
                    nc.tensor.matmul(pps[:, 0:256], dmy_a[:], dmy_b[:],
                                     start=True, stop=True)

            # ---- weight DMA (k path first - it gates the first scores) ---
            # Two half-granularity DMAs per weight: each dma_start pays
            # ~625ns on the single HWDGE descriptor-gen device, so fewer,
            # bigger transfers win; halves keep the prelude MM chain
            # streaming. Bulk loads go through gpsimd (SWDGE) which has its
            # own desc-gen path and leaves HWDGE free for the urgent loads.
            def wdma(w_sb, w_dr, n=2):
                r = w_dr.rearrange("(cc p) j -> p cc j", p=128)
                step = CCH // n
                for i in range(0, CCH, step):
                    nc.sync.dma_start(out=w_sb[:, i:i + step, :],
                                      in_=r[:, i:i + step, :])

            def load_x(tcx):
                tsl = ds(tcx * 512, 512)
                xtc = xp.tile([128, CCH, 512], bf16, tag="x")
                nc.sync.dma_start(out=xtc[:], in_=xT_r[:, :, tsl])
                return xtc

            def load_tabs(tcx, which, eng=None):
                tsl = ds(tcx * 512, 512)
                srcs = {"q": (("tqc", qcos), ("tqs", qsin)),
                        "k": (("tkc", kcos), ("tks", ksin))}[which]
                pool = qtabp if which == "q" else ktabp
                eng = eng or nc.sync
                tabs = []
                for nm, dr in srcs:
                    t = pool.tile([128, 512], bf16, tag=nm)
                    eng.dma_start(out=t[:], in_=dr[:, tsl])
                    tabs.append(t)
                return tabs

            def rotary(ps, tabs, dst, eng2=None):
                # dst = ps*cos + pairswap(ps)*sin   (sign folded into sin).
                # bf16 intermediates: the swap/t2/add legs are all-SBUF
                # 2-byte, so t2 and the add run in the DVE 2x mode.
                swp = rotp.tile([128, 512], fp32, tag="rt")
                nc.vector.stream_shuffle(swp[:], ps[:], SWAPM)
                t1 = rotp.tile([128, 512], bf16, tag="rt")
                nc.vector.tensor_tensor(t1[:], ps[:], tabs[0][:], MUL)
                t2 = rotp.tile([128, 512], bf16, tag="rt")
                (eng2 or nc.gpsimd).tensor_tensor(t2[:], swp[:], tabs[1][:], MUL)
                nc.vector.tensor_tensor(dst, t1[:], t2[:], ADD)

            # ---- prelude: x resident (all 4 chunks), k0/q0 for jt0 only;
            # everything else is injected into attention slots.
            held = {}

            def load_mask(q, half, eng=None):
                m = maskp.tile([128, 8, 512], bf16, tag="mask")
                (eng or nc.sync).dma_start(
                    out=m[:], in_=maskB_r[:, half * 8:half * 8 + 8,
                                          ds(q * 512, 512)])
                held[("mk", q, half)] = m

            # All prologue DMAs ride the sync HWDGE queue; transfers drain
            # the single DMA_ENGINES device in exactly this order. Sequence
            # by downstream need: first-scores path (x0,wk,ktabs0,qtabs0),
            # then mask half 0 (unblocks the slot-0 mask multiply on DVE),
            # x1 + wq, wv (v-tile pieces get pumped from slot ~1), k-tabs
            # and x chunks interleaved by deadline, mask half 1 before jt0's
            # tkt-8 multiply, wo/ident last.
            # interleave x0/wk piece DMAs so the k0 chain streams cc-by-cc
            xtc0 = xp.tile([128, CCH, 512], bf16, tag="x", name="xtc0")
            wk_r = wk.rearrange("(cc p) j -> p cc j", p=128)
            for i in range(0, CCH, 2):
                nc.sync.dma_start(out=xtc0[:, i:i + 2, :],
                                  in_=xT_r[:, i:i + 2, ds(0, 512)])
                nc.sync.dma_start(out=wk_sb[:, i:i + 2, :],
                                  in_=wk_r[:, i:i + 2, :])
            xall = [xtc0]
            ktabs = {0: load_tabs(0, "k")}
            wdma(wq_sb, wq, n=4)
            qtabs0 = load_tabs(0, "q")
            xall += [load_x(1)]
            ktabs[1] = load_tabs(1, "k")
            xall += [load_x(2)]
            ktabs[2] = load_tabs(2, "k")
            nc.sync.dma_start(
                out=wv_sb[:], in_=wv.rearrange("(cc p) j -> p cc j", p=128))
            xall += [load_x(3)]
            ktabs[3] = load_tabs(3, "k")
            load_mask(0, 0)
            load_mask(0, 1)
            nc.sync.dma_start(out=ident[:], in_=ident_d[:])
            nc.sync.dma_start(
                out=wo_sb[:], in_=wo.rearrange("(cc p) j -> p cc j", p=128))

            def proj_gen(w_sb, tcx, jt, fini):
                # generator piece: ~2 matmuls per step so injected work
                # never blocks the in-order PE stream for long
                ps = tailp.tile([128, 512], fp32, tag="tail")
                for h in range(4):
                    for cc in (2 * h, 2 * h + 1):
                        nc.tensor.matmul(ps[:], w_sb[:, cc, ts(jt, 128)],
                                         xall[tcx][:, cc, :],
                                         start=(cc == 0),
                                         stop=(cc == CCH - 1))
                    yield
                fini(ps)

            def kproj_gen(tcx, jt, eng2=None):
                return proj_gen(
                    wk_sb, tcx, jt,
                    lambda ps: rotary(ps[:], ktabs[tcx],
                                      kf[:, jt, ds(tcx * 512, 512)], eng2))

            def qproj_gen(tcx, tabs, jt, eng2=None):
                def fini(ps):
                    t = tabs["qt"] if isinstance(tabs, dict) else tabs
                    rotary(ps[:], t, qfq[:, tcx % 2, jt, :], eng2)
                return proj_gen(wq_sb, tcx, jt, fini)

            def kproj(tcx, jt, eng2=None):
                for _ in kproj_gen(tcx, jt, eng2):
                    pass

            def qproj(tcx, tabs, jt, eng2=None):
                for _ in qproj_gen(tcx, tabs, jt, eng2):
                    pass

            # jt0's chunk-0 projections in the prologue (k chain first: wk+x0
            # land before wq); jt1-3's ride early quarter-0 slots, which have
            # PE slack until PV kicks in at slot SKEW. Pacers bridge the PE
            # gap between the k chain and wq's arrival so the p-state ramp
            # never resets.
            kproj(0, 0, eng2=nc.vector)
            # the wq transfer is still in flight here; fill the window with
            # jt1-3's chunk-0 k chains instead of dummy pacers (their
            # rotaries also hide the first-scores wait on wq/qtabs0).
            kproj(0, 1, eng2=nc.vector)
            kproj(0, 2, eng2=nc.vector)
            kproj(0, 3, eng2=nc.vector)
            qproj(0, qtabs0, 0, eng2=nc.vector)

            # ---- injected side-work pieces -------------------------------
            def vtile_gen(tti_global):
                xt = xall[tti_global // 4]
                tsl = ts(tti_global % 4, 128)
                ps = tailp.tile([128, 8, 64], fp32, tag="tail")
                for h in range(4):
                    for cc in (2 * h, 2 * h + 1):
                        nc.tensor.matmul(ps[:, :, :], xt[:, cc, tsl],
                                         wv_sb[:, cc, :],
                                         start=(cc == 0),
                                         stop=(cc == CCH - 1))
                    yield
                nc.scalar.copy(vaug[:, tti_global, :, 0:64], ps[:, :, :])

            # ---- attention machinery -------------------------------------
            st = {"accs": None, "gkey": None, "y_written": 0}
            sideq = []                 # deferred small thunks (epilogue, wo)

            def acc_slice(accs, s):
                if s < 7:
                    return accs[0][:, s, :]
                return accs[1][:, s - 7, :]

            def epilogue(gkey, accs):
                tq4, jt = gkey
                rA = recp.tile([128, 8], fp32, tag="rec")
                nc.vector.reciprocal(rA[:, 0:7, None], accs[0][:, 0:7, 64:65])
                rB = recp.tile([128, 8], fp32, tag="rec")
                nc.vector.reciprocal(rB[:, 0:1, None], accs[1][:, 0:1, 64:65])

                # normalize all 7 pvA accumulators in ONE tensor_tensor with
                # the reciprocals broadcast along d (vs 7 serial TSPs); the
                # transposes then gather (s=qt, s=qt+4) as a strided slice.
                stg = stgp.tile([128, 8, 64], bf16, tag="stg")
                nc.vector.tensor_tensor(
                    stg[:, 0:7, :], accs[0][:, 0:7, 0:64],
                    rA[:, 0:7, None].broadcast_to((128, 7, 64)), MUL)
                nc.vector.tensor_scalar_mul(
                    stg[:, 7, :], accs[1][:, 0, 0:64], rB[:, 0:1])

                tb = tailp.tile([128, 4, 128], bf16, tag="tail")
                for qt in range(4):
                    nc.tensor.transpose(
                        tb[:, qt, :],
                        stg[:, 2 * qt:2 * qt + 2, :], ident[:])
                    nc.vector.tensor_copy(
                        oth[:, jt, ds(qt * 128, 128)], tb[:, qt, :])

            def wo_piece(tq4, jo):
                py = tailp.tile([128, 512], fp32, tag="tail")
                for cc in range(JT):
                    nc.tensor.matmul(py[:], wo_sb[:, cc, ts(jo, 128)],
                                     oth[:, cc, :],
                                     start=(cc == 0), stop=(cc == JT - 1))
                ysb = ysbp.tile([128, 512], fp32, tag="ysb")
                nc.vector.tensor_copy(ysb[:], py[:])
                nc.sync.dma_start(out=yT[ts(jo, 128), ds(tq4 * 512, 512)],
                                  in_=ysb[:])

            def ensure_group(gkey):
                if st["gkey"] == gkey:
                    return
                if st["gkey"] is not None:
                    epilogue(st["gkey"], st["accs"])
                    fin_tq4, fin_jt = st["gkey"]
                    if fin_jt == JT - 1:   # quarter's oth complete -> Wo
                        for jo in range(8):
                            sideq.append(
                                lambda tq4=fin_tq4, jo=jo: wo_piece(tq4, jo))
                pvA = pvp.tile([128, 7, 66], fp32, tag="pv")
                pvB = pvp.tile([128, 1, 66], fp32, tag="pv")
                st["accs"] = (pvA, pvB)
                st["gkey"] = gkey

            def emit_pv(pr, jt, tkt, gkey):
                ensure_group(gkey)
                for e in range(2):
                    h = jt * 2 + e
                    for qt in range(4):
                        s = qt * 2 + e
                        nc.tensor.matmul(
                            acc_slice(st["accs"], s),
                            pr[:, e, ts(qt, 128)],
                            vaug[:, tkt, h, 0:66],
                            start=(tkt == 0 and s in PV_START),
                            stop=(tkt == TT - 1 and s in PV_STOP),
                            skip_group_check=True)

            pvq = []                  # [(pr, jt, tkt, gkey, mb, r, slot), ...]
            workq = []                # [(key, generator)] fine-grained pieces

            def pop_pv():
                # DVE mask multiplies are emitted HERE, SKEW slots after the
                # exp: rotaries emitted near the exp slot never queue on DVE
                # behind a TT stalled on a mask DMA, and mask loads gain
                # ~20us of deadline slack. Pool-offloaded multiplies were
                # already emitted at exp time (their latency hides in the
                # skew and they never block DVE).
                pr, jt, tkt, gkey, mb, r, slot = pvq.pop(0)
                force(("v", tkt))           # vaug ready for pv
                if (gkey[0], slot) not in MASK_ON_GPSIMD:
                    nc.vector.tensor_tensor(
                        pr[:, :, :], pr[:, :, :],
                        mb[:, r, None, :].broadcast_to((128, 2, 512)), MUL)
                emit_pv(pr, jt, tkt, gkey)

            def pump(nsteps):
                while nsteps > 0 and workq:
                    key, g = workq[0]
                    try:
                        next(g)
                        nsteps -= 1
                    except StopIteration:
                        workq.pop(0)

            def force(key):
                # emission-order deadline: finish FIFO head pieces up to and
                # including `key` so dependent reads are emitted after writes
                while any(k == key for k, _ in workq):
                    k0, g = workq[0]
                    try:
                        next(g)
                    except StopIteration:
                        workq.pop(0)

            # ---- main loop ------------------------------------------------

            for tq4 in range(NQ):
                qsl = ds(tq4 * 512, 512)
                if tq4 > 0:
                    load_mask(tq4, 1)   # first half was prefetched

                inject = {}

                def add(slot, fn):
                    inject.setdefault(slot, []).append(fn)

                def addg(slot, key, mk):
                    # queue a generator piece at slot; tracked for deadlines
                    def starter():
                        workq.append((key, mk()))
                    add(slot, starter)

                def addk(slot, tcx, jt):
                    addg(slot, ("k", tcx, jt), lambda: kproj_gen(tcx, jt))

                def addq(slot, tcx, jt, tabs):
                    addg(slot, ("q", tcx, jt),
                         lambda: qproj_gen(tcx, tabs, jt))

                def addv(slot, i):
                    addg(slot, ("v", i), lambda: vtile_gen(i))

                if tq4 == 0:
                    # remaining k chunks per jt (deadline slot jt*16+4c),
                    # k0/q0 projections for jt 1-3 (deadline jt*16), all 16
                    # v tiles (deadline tt+SKEW), q quarter 1 late.
                    # workq is FIFO and pump pops the head only, so adds are
                    # sequenced by DMA arrival: x0/wq-based first, then the
                    # x1/x2/x3-gated chunks as close to their force slot as
                    # possible so a not-yet-landed DMA never wedges the head.
                    addk(0, 1, 0)                    # x1-based
                    addq(3, 0, 1, qtabs0)
                    addk(4, 2, 0)                    # x2-based
                    addq(7, 0, 2, qtabs0)
                    addv(7, 0)                       # wv-gated
                    addk(9, 3, 0)                    # x3-based
                    addv(9, 1)
                    addq(11, 0, 3, qtabs0)
                    addv(12, 2)
                    addv(13, 3)
                    addk(14, 1, 1)
                    addv(15, 4)
                    addk(16, 2, 1)
                    addv(17, 5)
                    addk(18, 3, 1)
                    addv(19, 6)
                    addv(20, 7)
                    addv(22, 8)                      # v8-11 (x2-based)
                    addv(23, 9)
                    addv(24, 10)
                    addv(25, 11)
                    addv(26, 12)                     # v12-15 (x3-based)
                    addv(27, 13)
                    addv(28, 14)
                    addv(29, 15)
                    addk(31, 1, 2)
                    addk(34, 2, 2)
                    addk(37, 3, 2)
                    addk(44, 1, 3)
                    addk(46, 2, 3)
                    addk(48, 3, 3)
                    add(50, lambda: held.__setitem__("qt", load_tabs(1, "q")))
                    for jt in range(JT):
                        addq(52 + 3 * jt, 1, jt, held)
                elif tq4 < NQ - 1:
                    add(24, lambda tq4=tq4: held.__setitem__(
                        "qt", load_tabs(tq4 + 1, "q")))
                    for jt in range(JT):
                        addq(28 + 3 * jt, tq4 + 1, jt, held)
                if tq4 < NQ - 1:
                    add(44, lambda tq4=tq4: load_mask(tq4 + 1, 0))

                for jt in range(JT):
                    for tkt in range(TT):
                        slot = jt * 16 + tkt
                        if tkt == 0:
                            force(("q", tq4, jt))   # qfq ready for scores
                        if tkt % 4 == 0:
                            force(("k", tkt // 4, jt))  # kf ready for scores
                        elif tkt % 4 == 1 and tkt < 13:
                            # drain the NEXT chunk 3 slots early so its
                            # 8-MM chain + rotary latency is hidden
                            force(("k", tkt // 4 + 1, jt))
                        ps = scp.tile([128, 2, 512], fp32, tag="sc")
                        mb = held[("mk", tq4, tkt // 8)]
                        r = tkt % 8
                        for e in range(2):
                            nc.tensor.matmul(
                                ps[:, e, :],
                                kf[ds(e * 64, 64), jt, ts(tkt, 128)],
                                qfq[ds(e * 64, 64), tq4 % 2, jt, :],
                                start=True, stop=True)
                        pump(2)
                        if tq4 == NQ - 1 and slot >= 40:
                            target = 2 if slot >= 56 else 8
                        elif slot >= 56:
                            target = 14   # taper into the quarter boundary
                        else:
                            target = SKEW
                        while len(pvq) >= target:
                            pop_pv()
                        pump(2)
                        pr = prp.tile([128, 2, 512], bf16, tag="pr")
                        nc.scalar.activation(pr[:, :, :], ps[:, :, :],
                                             EXP, bias=bias_m40[:, :],
                                             scale=0.03125)
                        if (tq4, slot) in MASK_ON_GPSIMD:
                            nc.gpsimd.tensor_tensor(
                                pr[:, :, :], pr[:, :, :],
                                mb[:, r, None, :].broadcast_to((128, 2, 512)),
                                MUL)
                        pvq.append((pr, jt, tkt, (tq4, jt), mb, r, slot))
                        for _ in range(2 if tq4 == NQ - 1 else 1):
                            if sideq:
                                sideq.pop(0)()
                        for fn in inject.get(slot, ()):
                            fn()
                        pump(2)
                        if PACER_EVERY and tq4 > 0 and slot % PACER_EVERY == 0:
                            pacer(1)
                while workq:      # quarter boundary: flush queued pieces
                    pump(100)

            # ---- drain ----------------------------------------------------
            while pvq:
                pop_pv()
            epilogue(st["gkey"], st["accs"])
            while sideq:
                sideq.pop(0)()
            for jo in range(8):
                wo_piece(NQ - 1, jo)
    nc.finalize()
    return nc


def _host_inputs(x, attn_mask, Wq, Wk, Wv, Wo):
    x = np.asarray(x, dtype=np.float32)
    attn_mask = np.asarray(attn_mask)
    Wq = np.asarray(Wq, dtype=np.float32)
    Wk = np.asarray(Wk, dtype=np.float32)
    Wv = np.asarray(Wv, dtype=np.float32)
    Wo = np.asarray(Wo, dtype=np.float32)

    cos, sin, scale = _rot_tables_np()
    cosT, sinT, scaleT = cos.T, sin.T, scale.T            # [D, T]
    # sign-fold for the partition-swap rotate-half: even d rows get -sin
    sgn = np.where(np.arange(D) % 2 == 0, -1.0, 1.0).astype(np.float32)[:, None]
    qcos = np.ascontiguousarray(np.tile(cosT * scaleT, (2, 1))
                                .astype(ml_dtypes.bfloat16))
    qsin = np.ascontiguousarray(np.tile(sinT * scaleT * sgn, (2, 1))
                                .astype(ml_dtypes.bfloat16))
    kcos = np.ascontiguousarray(np.tile(cosT / scaleT, (2, 1))
                                .astype(ml_dtypes.bfloat16))
    ksin = np.ascontiguousarray(np.tile(sinT / scaleT * sgn, (2, 1))
                                .astype(ml_dtypes.bfloat16))
    ident = np.eye(128, dtype=ml_dtypes.bfloat16)

    in_maps = []
    for b in range(B):
        xTb = np.ascontiguousarray(x[b].T.astype(ml_dtypes.bfloat16))
        mB16 = np.ascontiguousarray(
            (attn_mask[b, 0].T != 0).astype(ml_dtypes.bfloat16))  # {0, 1}
        for g in range(G):
            sl = slice(CG * g, CG * (g + 1))
            Wq_g, Wk_g, Wv_g = Wq[sl], Wk[sl], Wv[sl]
            in_maps.append({
                "xT": xTb,
                "maskB": mB16,
                "wq": np.ascontiguousarray(Wq_g.T.astype(ml_dtypes.bfloat16)),
                "wk": np.ascontiguousarray(Wk_g.T.astype(ml_dtypes.bfloat16)),
                "wv": np.ascontiguousarray(Wv_g.T.astype(ml_dtypes.bfloat16)),
                "wo": np.ascontiguousarray(Wo[:, sl].T.astype(ml_dtypes.bfloat16)),
                "qcos": qcos, "qsin": qsin, "kcos": kcos, "ksin": ksin,
                "ident": ident,
            })
    return in_maps


def kernel(x, attn_mask, Wq, Wk, Wv, Wo):
    from concourse.bass_utils import run_bass_kernel_spmd

    if "nc" not in _CACHE:
        _CACHE["nc"] = _build_bass()
    nc = _CACHE["nc"]

    in_maps = _host_inputs(x, attn_mask, Wq, Wk, Wv, Wo)
    res = run_bass_kernel_spmd(nc, in_maps, core_ids=list(range(NCORES)))
    _CACHE["last_results"] = res

    y = np.empty((B, T, C), dtype=np.float32)
    for b in range(B):
        acc = np.asarray(res.results[2 * b]["yT"], dtype=np.float32) + \
              np.asarray(res.results[2 * b + 1]["yT"], dtype=np.float32)
        y[b] = acc.T
    return y

